# revision 49
# baseline (speedup 1.0000x reference)
"""Trainium2 Bass kernel for nn_D2GroupConvolutionLayer (D2-equivariant GAT).

Math: for output view g and input view h the layer is a GAT with a GLOBAL
softmax over edges.  Since score(e) = u[src] + v[dst], the pipeline collapses
to dense algebra per (batch, g, h):

    out += diag(b) . M . diag(a) . X2 . W / (V * b^T M a)

with a = exp(u - max u), b = exp(v - max v) per-node scalars, X2 = [x_h, x_gh]
the sign-aligned feature pair [N, 2F], and M[d, s] the {0,1,2} edge
multiplicity matrix (self-loops included).

Key optimizations vs the straightforward M @ (diag(a) X2 W) order:
  * Re-associate: K' = M @ (diag(a) X2) (257-wide rhs instead of 513), then
    G = K' @ W.  The big M-contraction runs as fp8e4 DoubleRow matmuls at 2x
    rate: M is EXACT in fp8 ({0,1,2}) and (a*X2) is fed as an exact hi+lo
    fp8 pair (hi = fp8(ax), lo = fp8(ax - hi), ~7 mantissa bits combined)
    accumulated in one PSUM group -> bf16-grade accuracy, double throughput.
  * Score path computed TRANSPOSED: H^T = W^T X2^T so that the att
    dot-products become tiny PE matvecs (lhsT = att columns) instead of
    128 wide DVE reductions.  lrelu(x) = 0.2x + 0.8relu(x): the relu runs
    on the Activation engine (doubling as the PSUM->SBUF copy); the linear
    0.2 * X2 (W att_j) term is a tiny [N, 2] per-view tensor from the host.
    u,v come back to node-major layout via one transpose-DMA each.
  * Cross-partition max/sum run as GPSIMD partition_all_reduce (no PE
    broadcast matmuls, no round-trip DMAs).
  * z comes from an fp8 a-column contracted d-major (out free = 1, ~free).

Sharding: data-parallel over the 8 (batch b, output view g) pairs, one
NeuronCore each; no cross-core communication.  One SPMD program: all
(b, g)-dependence is baked into per-core input tensors on the host (pure
relayout + sign flips + the tiny l-term).
"""

import sys
from contextlib import ExitStack

for _p in ("/opt/trn_rl_repo/concourse", "/opt/trn_rl_repo"):
    if _p not in sys.path:
        sys.path.insert(0, _p)

import ml_dtypes  # noqa: E402
import numpy as np  # noqa: E402

import concourse.bass as bass  # noqa: E402
import concourse.bacc as bacc  # noqa: E402
import concourse.bass_isa as bass_isa  # noqa: E402
import concourse.mybir as mybir  # noqa: E402
import concourse.tile as tile  # noqa: E402
import concourse.tile_utils as tile_utils  # noqa: E402
import bass_rust  # noqa: E402

# Problem constants (hardcoded per harness contract).
B, V, N, F, O = 2, 4, 2048, 128, 512
NT = N // 128  # node tiles
F2 = 2 * F  # 256 pair features
AXW = 272  # axq row stride (256 feats + a col + pad to %16)
F32, BF16 = mybir.dt.float32, mybir.dt.bfloat16
FP8 = mybir.dt.float8e4
DR = mybir.MatmulPerfMode.DoubleRow
MUL = mybir.AluOpType.mult
ADD = mybir.AluOpType.add
SUB = mybir.AluOpType.subtract
MAX = mybir.AluOpType.max
RELU = mybir.ActivationFunctionType.Relu
EXP = mybir.ActivationFunctionType.Exp

tile_utils.max_sbuf_usage = 207 * 1024

# Skip the lo-residual pass for the first LO_SKIP s-tiles: trades a little
# quantization error (measured 4.3e-3 -> 1.16e-2 at LO_SKIP=2, gate 2e-2)
# for 2/16 of the DoubleRow matmul work.
LO_SKIP = 2


def _chain(f, g):
    if f is None:
        return g
    return lambda: (f(), g())


class _TileContext(tile.TileContext):
    """Splits the exit-drain's sem waits across single-wait carrier nops.

    Walrus caps sync waits at 1/instruction; the stock _drain_and_barrier
    attaches every outstanding DMA/engine sem wait to one Drain and fails
    codegen with "Too many sync wait commands".
    """

    def _drain_and_barrier(self, tick_clock, wait_clock):
        nc = self.nc
        probe = nc.sync.nop(nofuse=True)
        wait_clock.add_sem_waits(
            probe.ins, bass_rust.ScopedClock({None: tick_clock.global_clock})
        )
        si = probe.ins.sync_info
        if si is not None and si.on_wait and len(si.on_wait) > 1:
            waits = list(si.on_wait)
            si.on_wait = [waits[0]]
            for w in waits[1:]:
                carrier = nc.sync.nop(nofuse=True)
                carrier.ins.sync_info = mybir.SyncInfo(on_wait=[w], on_update=[])
        nc.sync.drain()
        nc.all_engine_barrier()
        popped = nc._tile_sem_poison_stack.pop()
        assert popped is self._sem_poison
        nc.clear_and_free_semaphores(list(self.sems.allocated().values()))
        nc.all_engine_barrier()


def _build_program():
    nc = bacc.Bacc("TRN2", target_bir_lowering=False, debug=False)

    sxst_d = nc.dram_tensor("sxst", [V, 128, 2, N], BF16, kind="ExternalInput").ap()
    sxp_d = nc.dram_tensor("sxp", [V, 128, NT * F2], BF16, kind="ExternalInput").ap()
    mt_d = nc.dram_tensor("mt", [4, 128, NT * 512], FP8, kind="ExternalInput").ap()
    wab_d = nc.dram_tensor("wab", [128, 2, O + 8], BF16, kind="ExternalInput").ap()
    lps_d = nc.dram_tensor("lps", [128, V * NT * 2], F32, kind="ExternalInput").ap()
    out_d = nc.dram_tensor("out", [NT, 128, O], F32, kind="ExternalOutput").ap()
    urd_d = nc.dram_tensor("urd", [V, 2, N], F32, kind="Internal").ap()
    import os as _os
    dbg = _os.environ.get("KDBG") == "1"
    if dbg:
        dbg_d = {
            name: nc.dram_tensor(
                name, shape, BF16 if name == "d_kpt" else F32,
                kind="ExternalOutput",
            ).ap()
            for name, shape in {
                "d_urow": [2, N], "d_us": [128, NT], "d_vs": [128, NT],
                "d_a": [128, NT], "d_b": [128, NT], "d_ma": [128, NT],
                "d_kpt": [128, 2 * N], "d_bsc": [128, NT],
            }.items()
        }

    with ExitStack() as ctx:
        tc = ctx.enter_context(_TileContext(nc))
        pool = ctx.enter_context(tc.tile_pool(name="main", bufs=1))
        stp = ctx.enter_context(tc.tile_pool(name="st", bufs=2))
        axp = ctx.enter_context(tc.tile_pool(name="ax", bufs=4))
        ktp = ctx.enter_context(tc.tile_pool(name="kt", bufs=2))
        ltp = ctx.enter_context(tc.tile_pool(name="lt", bufs=8))
        tmp = ctx.enter_context(tc.tile_pool(name="tmq", bufs=2))
        urdp = ctx.enter_context(tc.tile_pool(name="urd", bufs=2, space="DRAM"))
        php = ctx.enter_context(tc.tile_pool(name="ph", bufs=2, space="PSUM"))
        pkp = ctx.enter_context(tc.tile_pool(name="pk", bufs=2, space="PSUM"))
        pgp = ctx.enter_context(tc.tile_pool(name="pg", bufs=3, space="PSUM"))
        pqp = ctx.enter_context(tc.tile_pool(name="pq", bufs=1, space="PSUM"))

        # ---- persistent SBUF ----
        sxst = pool.tile([128, V, 2, N], BF16)  # f-major x^T pairs per view
        sxp = pool.tile([128, V, NT, F2], BF16)  # s-major x pairs per view
        mt = pool.tile([128, NT, N], FP8)  # mt[p, t, d] = M[d, t*128+p]
        wab = pool.tile([128, 2, O + 8], BF16)  # [W half | 0.8*att o-major]
        lps = pool.tile([128, V, NT, 2], F32)  # 0.2 * X2 @ (W att_j)
        out_acc = pool.tile([128, NT, O], F32)

        # Input DMA is STAGED through the pipeline: the cost model (and HW
        # ring occupancy) serializes transfers, so bulk loads are emitted
        # just-in-time to keep the rings free for the small latency-critical
        # stats transpose-DMAs.  Here: only what the prologue needs.
        nc.sync.dma_start(wab[:], wab_d[:])
        for c in range(4):  # view-0 f-major pair, chunked so A0 starts early
            cs = slice(c * 512, (c + 1) * 512)
            nc.sync.dma_start(sxst[:, 0, :, cs], sxst_d[0, :, :, cs])
        nc.sync.dma_start(lps[:], lps_d[:])
        nc.sync.dma_start(sxp[:, 0], sxp_d[0])
        nc.sync.dma_start(sxst[:, 1], sxst_d[1])

        def dma_mt(c):
            nc.sync.dma_start(mt[:, :, c * 512 : (c + 1) * 512], mt_d[c])

        def dma_sxst(v):
            nc.sync.dma_start(sxst[:, v], sxst_d[v])

        def dma_sxp(v):
            nc.sync.dma_start(sxp[:, v], sxp_d[v])

        st = {}

        def a_phase(h):
            """H^T = W^T X2^T per s-chunk; relu; u,v via PE matvecs."""
            urow = stp.tile([2, N], F32, tag="urow", name=f"urow{h}")
            rts = {}

            def h_chunk(c):
                for ot in range(4):
                    ph_t = php.tile([128, 512], F32, tag="ph", name=f"ph{h}_{c}_{ot}")
                    for i in range(2):
                        nc.tensor.matmul(
                            ph_t[:], wab[:, i, bass.ts(ot, 128)],
                            sxst[:, h, i, bass.ts(c, 512)],
                            start=(i == 0), stop=(i == 1),
                        )
                    rt = ltp.tile([128, 512], BF16, tag="rt", name=f"rt{h}_{c}_{ot}")
                    if ot % 2 == 1:  # relus split ACT/DVE so PE sets the pace
                        nc.vector.tensor_scalar(rt[:], ph_t[:], 0.0, None, op0=MAX)
                    else:
                        nc.scalar.activation(rt[:], ph_t[:], RELU)
                    rts[(c, ot)] = rt

            def mv_chunk(c):
                pu = pqp.tile([128, 512], F32, tag="pq", name=f"pu{h}_{c}")
                for ot in range(4):
                    nc.tensor.matmul(
                        pu[0:2, :], wab[:, 0:2, O + ot], rts[(c, ot)][:],
                        start=(ot == 0), stop=(ot == 3),
                    )
                nc.scalar.copy(urow[0:2, bass.ts(c, 512)], pu[0:2, :])

            h_chunk(0)
            for c in range(1, 4):
                h_chunk(c)
                mv_chunk(c - 1)
            mv_chunk(3)
            st[h] = [urow]

        def stats_tr(h):
            """u,v back to node-major via transpose-DMA; + linear term;
            global max via partition_all_reduce; a,b = exp(. - max)."""
            urow = st[h][0]
            us = stp.tile([128, NT], F32, tag="us", name=f"us{h}")
            vs = stp.tile([128, NT], F32, tag="vs", name=f"vs{h}")
            # node-major transpose via DRAM bounce (SBUF APs cannot take a
            # partition dim from free strides; DRAM APs are unconstrained).
            # The scratch is a DRAM *pool tile* so the RAW dep is tracked.
            urd_t = urdp.tile([2, N], F32, tag="urd", name=f"urd{h}")
            nc.sync.dma_start(urd_t[:], urow[0:2, :])
            for j, dst in ((0, us), (1, vs)):
                src = urd_t[j : j + 1, :].rearrange(
                    "a (c t p) -> (a p) (c t)", c=4, t=4, p=128
                )
                nc.sync.dma_start(dst[:], src)
                nc.vector.tensor_tensor(dst[:], dst[:], lps[:, h, :, j], op=ADD)
            mstat = stp.tile([128, 2], F32, tag="mst", name=f"mst{h}")
            negm = stp.tile([128, 2], F32, tag="negm", name=f"negm{h}")
            nc.vector.reduce_max(mstat[:, 0:1], us[:], axis=mybir.AxisListType.X)
            nc.vector.reduce_max(mstat[:, 1:2], vs[:], axis=mybir.AxisListType.X)
            nc.gpsimd.partition_all_reduce(
                negm[:], mstat[:], 128, bass_isa.ReduceOp.max
            )
            nc.scalar.mul(negm[:], negm[:], -1.0)
            st[h].extend([us, vs, negm])

        def quant(h, t0, t1):
            """a,b = exp; axq hi/lo fp8 pair of a*X2 for tiles [t0, t1)."""
            if t0 == 0:
                us, vs, negm = st[h][1:4]
                a_st = stp.tile([128, NT], F32, tag="ast", name=f"ast{h}")
                b_st = stp.tile([128, NT], F32, tag="bst", name=f"bst{h}")
                nc.scalar.activation(a_st[:], us[:], EXP, bias=negm[:, 0:1])
                nc.scalar.activation(b_st[:], vs[:], EXP, bias=negm[:, 1:2])
                hi = axp.tile([128, NT, AXW], FP8, tag="hi", name=f"hi{h}")
                lo = axp.tile([128, NT, AXW], FP8, tag="lo", name=f"lo{h}")
                nc.vector.tensor_copy(hi[:, :, F2], a_st[:])  # a col (hi only)
                st[h].extend([a_st, b_st, hi, lo])
            a_st, _, hi, lo = st[h][4:8]
            for t in range(t0, t1):
                if t % 2 == 0:
                    nc.scalar.mul(
                        hi[:, t, 0:F2], sxp[:, h, t, :], a_st[:, t : t + 1]
                    )
                else:
                    nc.vector.tensor_scalar(
                        hi[:, t, 0:F2], sxp[:, h, t, :], a_st[:, t : t + 1],
                        None, op0=MUL,
                    )
            for t in range(max(t0, LO_SKIP), t1):
                nc.vector.scalar_tensor_tensor(
                    lo[:, t, 0:F2], sxp[:, h, t, :], a_st[:, t : t + 1],
                    hi[:, t, 0:F2], op0=MUL, op1=SUB,
                )

        def g_tile(hp, d):
            """One G2 d-tile of view hp: 2 bf16 matmuls + scale-accumulate."""
            kpt, bsc = st[hp][8], st[hp][10]
            pg_t = pgp.tile([128, O], F32, tag="pg", name=f"pg{hp}_{d}")
            nc.tensor.matmul(
                pg_t[:], kpt[:, 0, bass.ts(d, 128)], wab[:, 0, 0:O],
                start=True, stop=False,
            )
            nc.tensor.matmul(
                pg_t[:], kpt[:, 1, bass.ts(d, 128)], wab[:, 1, 0:O],
                start=False, stop=True,
            )
            if hp == 0:
                nc.scalar.mul(out_acc[:, d, :], pg_t[:], bsc[:, d : d + 1])
            elif hp < 3 or d % 2 == 0:
                nc.vector.scalar_tensor_tensor(
                    out_acc[:, d, :], pg_t[:], bsc[:, d : d + 1],
                    out_acc[:, d, :], op0=MUL, op1=ADD,
                )
            else:  # tail: ACT mul + Pool add keeps the epilogue off DVE
                tq = tmp.tile([128, O], BF16, tag="tq", name=f"tq{d}")
                nc.scalar.mul(tq[:], pg_t[:], bsc[:, d : d + 1])
                nc.gpsimd.tensor_tensor(
                    out_acc[:, d, :], tq[:], out_acc[:, d, :], op=ADD
                )
            if hp == V - 1:
                nc.sync.dma_start(out_d[d], out_acc[:, d, :])

        def ma_pass(h, pma, d0, d1):
            """ma columns [d0,d1) as free=1 DoubleRow matvecs (~free on PE).
            One PSUM accumulation group spans all 16 columns of the bank."""
            hi = st[h][6]
            for d in range(d0, d1):
                for t in range(8):
                    nc.tensor.matmul(
                        pma[:, d : d + 1],
                        mt[:, 2 * t : 2 * t + 2, bass.ts(d, 128)],
                        hi[:, 2 * t : 2 * t + 2, F2 : F2 + 1],
                        start=(d == 0 and t == 0), stop=(d == NT - 1 and t == 7),
                        perf_mode=DR, skip_group_check=True,
                    )

        def k_phase(h, hp, hooks=None, ma_first=False, gq2=None):
            """K'^T = M @ (a*X2) via hi+lo fp8 DoubleRow; ma = M @ a8 d-major.
            Interleaves the G2 pass of view hp and per-group hook callbacks."""
            hi, lo = st[h][6], st[h][7]
            kpt = ktp.tile([128, 2, N], BF16, tag="kpt", name=f"kpt{h}")
            ma_sb = stp.tile([128, NT], F32, tag="ma", name=f"ma{h}")
            pma = pqp.tile([128, 512], F32, tag="pq", name=f"pma{h}")
            st[h].append(kpt)
            st[h].append(ma_sb)
            if ma_first:
                ma_pass(h, pma, 0, NT)
                nc.vector.tensor_copy(ma_sb[:], pma[:, 0:NT])
                z_chain(h)
            gq = list(range(NT)) if hp is not None else []
            gi = 0
            for c in range(4):
                for ft in range(2):
                    pk_t = pkp.tile(
                        [128, 512], F32, tag="pk", name=f"pk{h}_{c}_{ft}"
                    )
                    fc = slice(ft * 128, (ft + 1) * 128)
                    dc = slice(c * 512, (c + 1) * 512)
                    for t in range(8):
                        nc.tensor.matmul(
                            pk_t[:], hi[:, 2 * t : 2 * t + 2, fc],
                            mt[:, 2 * t : 2 * t + 2, dc],
                            start=(t == 0), stop=False, perf_mode=DR,
                        )
                    for t in range(LO_SKIP // 2, 8):
                        nc.tensor.matmul(
                            pk_t[:], lo[:, 2 * t : 2 * t + 2, fc],
                            mt[:, 2 * t : 2 * t + 2, dc],
                            start=False, stop=(t == 7), perf_mode=DR,
                        )
                    nc.scalar.copy(kpt[:, ft, dc], pk_t[:])
                    if not ma_first:
                        d = c * 4 + ft * 2
                        ma_pass(h, pma, d, d + 2)
                    if hooks and gi in hooks:
                        hooks[gi]()
                    for _ in range(4 if gq2 is not None else 2):
                        if gq:
                            g_tile(hp, gq.pop(0))
                    # own-view G2 tiles once this chunk's kpt columns landed
                    if gq2 is not None:
                        avail = (c + ft) * 4
                        for _ in range(4):
                            if gq2 and gq2[0] < avail:
                                g_tile(h, gq2.pop(0))
                    gi += 1
            if not ma_first:
                nc.vector.tensor_copy(ma_sb[:], pma[:, 0:NT])

        def z_chain(h):
            """z = sum_d b_d ma_d via partition_all_reduce; bsc = b/(V z)."""
            b_st, ma_sb = st[h][5], st[h][9]
            zscr = stp.tile([128, NT], F32, tag="zscr", name=f"zscr{h}")
            zp = stp.tile([128, 1], F32, tag="zp", name=f"zp{h}")
            zs = stp.tile([128, 1], F32, tag="zs", name=f"zs{h}")
            nc.vector.scalar_tensor_tensor(
                zscr[:], ma_sb[:], 1.0, b_st[:], op0=MUL, op1=MUL,
                accum_out=zp[:],
            )
            nc.gpsimd.partition_all_reduce(
                zs[:], zp[:], 128, bass_isa.ReduceOp.add
            )
            nc.vector.reciprocal(zs[:], zs[:])
            nc.vector.tensor_scalar(zs[:], zs[:], 1.0 / V, None, op0=MUL)
            bsc = stp.tile([128, NT], F32, tag="bsc", name=f"bsc{h}")
            nc.vector.tensor_scalar(bsc[:], b_st[:], zs[:, 0:1], None, op0=MUL)
            st[h].append(bsc)

        def q_hooks(h):
            """Spread view h's exp+quant across the previous k_phase."""
            return {
                0: lambda: quant(h, 0, 6),
                1: lambda: quant(h, 6, 11),
                2: lambda: quant(h, 11, 16),
            }

        # ---- software pipeline over views ----
        # st[h]: 0 urow, 1 us, 2 vs, 3 negm, 4 a, 5 b, 6 hi, 7 lo,
        #        8 kpt, 9 ma, 10 bsc
        a_phase(0)
        stats_tr(0)
        dma_mt(0)
        dma_sxst(2)
        a_phase(1)
        stats_tr(1)
        dma_mt(1)
        dma_sxp(1)
        quant(0, 0, 16)
        k0_hooks = q_hooks(1)
        k0_hooks[1] = _chain(k0_hooks.get(1), lambda: dma_mt(2))
        k0_hooks[3] = _chain(k0_hooks.get(3), lambda: dma_mt(3))
        k0_hooks[5] = _chain(k0_hooks.get(5), lambda: dma_sxp(2))
        k0_hooks[7] = _chain(k0_hooks.get(7), lambda: dma_sxst(3))
        k_phase(0, None, k0_hooks)
        a_phase(2)
        stats_tr(2)
        z_chain(0)
        k1_hooks = q_hooks(2)
        k1_hooks[4] = _chain(k1_hooks.get(4), lambda: dma_sxp(3))
        k_phase(1, 0, k1_hooks)
        a_phase(3)
        stats_tr(3)
        z_chain(1)
        k_phase(2, 1, q_hooks(3))
        z_chain(2)
        gq3 = list(range(NT))
        k_phase(3, 2, ma_first=True, gq2=gq3)
        while gq3:
            g_tile(3, gq3.pop(0))

        if dbg:
            hdbg = 3  # view-3 tiles are still live at program end
            nc.sync.dma_start(dbg_d["d_urow"][:], st[hdbg][0][0:2, :])
            nc.sync.dma_start(dbg_d["d_us"][:], st[hdbg][1][:])
            nc.sync.dma_start(dbg_d["d_vs"][:], st[hdbg][2][:])
            nc.sync.dma_start(dbg_d["d_a"][:], st[hdbg][4][:])
            nc.sync.dma_start(dbg_d["d_b"][:], st[hdbg][5][:])
            nc.sync.dma_start(dbg_d["d_ma"][:], st[hdbg][9][:])
            nc.sync.dma_start(dbg_d["d_bsc"][:], st[hdbg][10][:])
            nc.sync.dma_start(dbg_d["d_kpt"][:], st[hdbg][8][:])

    nc.compile()
    return nc


_SIGNS = None


def _signs():
    global _SIGNS
    if _SIGNS is None:
        s = np.ones((4, F), dtype=np.float32)
        for r in range(4):
            if r & 1:
                s[r, [0, 2]] = -1.0
            if r & 2:
                s[r, [1, 3]] = -1.0
        _SIGNS = s
    return _SIGNS


def _host_prep(x, edge_index, W, att, bias):
    """Per-core relayout + sign flips + the tiny O(N*F) l-term."""
    signs = _signs()
    x = np.ascontiguousarray(x, dtype=np.float32)
    W = np.asarray(W, dtype=np.float32)
    att = np.asarray(att, dtype=np.float32).reshape(2 * O)
    ei = np.asarray(edge_index)

    # mt[p, t, d] = M[d, t*128 + p]; entries {0,1,2} exact in fp8e4
    M = np.zeros((N, N), dtype=np.float32)
    np.add.at(M, (ei[1], ei[0]), 1.0)
    M[np.arange(N), np.arange(N)] += 1.0
    MT = np.ascontiguousarray(M.T).reshape(NT, 128, N).transpose(1, 0, 2)
    mt_t = MT.astype(ml_dtypes.float8_e4m3)  # [128, NT, N]
    mt_chunks = np.ascontiguousarray(
        mt_t.reshape(128, NT, 4, 512).transpose(2, 0, 1, 3).reshape(4, 128, NT * 512)
    )

    # wab[p, j, 0:O] = W[j*128+p, :]; wab[p, j, O+ot] = 0.8*att_j[ot*128+p]
    wab = np.zeros((128, 2, O + 8), dtype=np.float32)
    wab[:, :, :O] = W.reshape(2, 128, O).transpose(1, 0, 2)
    wab[:, 0, O : O + 4] = 0.8 * att[:O].reshape(4, 128).T
    wab[:, 1, O : O + 4] = 0.8 * att[O:].reshape(4, 128).T
    wab = np.ascontiguousarray(wab).astype(ml_dtypes.bfloat16)
    wa = 0.2 * np.stack([W @ att[:O], W @ att[O:]], axis=1)  # [256, 2]

    in_maps = []
    for core in range(8):
        b, g = divmod(core, V)
        # sign-aligned per-view copies: sxs_v = x[b, v] * signs[v ^ g]
        sxs = np.empty((V, N, F), dtype=np.float32)
        for v in range(V):
            sxs[v] = x[b, v] * signs[v ^ g]
        sxst = np.empty((V, 128, 2, N), dtype=np.float32)  # f-major halves
        sxp = np.empty((V, 128, NT, F2), dtype=np.float32)  # s-major pairs
        lps = np.empty((V, 128, NT, 2), dtype=np.float32)  # linear score term
        for h in range(V):
            gh = g ^ h
            sxst[h, :, 0] = sxs[h].T
            sxst[h, :, 1] = sxs[gh].T
            pair = np.concatenate([sxs[h], sxs[gh]], axis=1)  # [N, 256]
            sxp[h] = pair.reshape(NT, 128, F2).transpose(1, 0, 2)
            lps[h] = (pair @ wa).reshape(NT, 128, 2).transpose(1, 0, 2)
        in_maps.append(
            {
                "sxst": sxst.astype(ml_dtypes.bfloat16),
                "sxp": np.ascontiguousarray(
                    sxp.reshape(V, 128, NT * F2)
                ).astype(ml_dtypes.bfloat16),
                "mt": mt_chunks,
                "wab": wab,
                "lps": np.ascontiguousarray(
                    lps.transpose(1, 0, 2, 3).reshape(128, V * NT * 2)
                ),
            }
        )
    return in_maps


_NC = None


def kernel(x, edge_index, W, att, bias):
    global _NC
    if _NC is None:
        _NC = _build_program()
    in_maps = _host_prep(x, edge_index, W, att, bias)

    from concourse.bass_utils import run_bass_kernel_spmd

    res = run_bass_kernel_spmd(_NC, in_maps, list(range(8)))
    out = np.empty((B, V, N, O), dtype=np.float32)
    for core in range(8):
        b, g = divmod(core, V)
        out[b, g] = res.results[core]["out"].reshape(N, O)
    bias = np.asarray(bias, dtype=np.float32)
    if np.any(bias):
        out += bias  # bias is zero for this problem's inputs
    return out


# revision 53
# speedup vs baseline: 1.0134x; 1.0134x over previous
"""Trainium2 Bass kernel for nn_D2GroupConvolutionLayer (D2-equivariant GAT).

Math: for output view g and input view h the layer is a GAT with a GLOBAL
softmax over edges.  Since score(e) = u[src] + v[dst], the pipeline collapses
to dense algebra per (batch, g, h):

    out += diag(b) . M . diag(a) . X2 . W / (V * b^T M a)

with a = exp(u - max u), b = exp(v - max v) per-node scalars, X2 = [x_h, x_gh]
the sign-aligned feature pair [N, 2F], and M[d, s] the {0,1,2} edge
multiplicity matrix (self-loops included).

Key optimizations vs the straightforward M @ (diag(a) X2 W) order:
  * Re-associate: K' = M @ (diag(a) X2) (257-wide rhs instead of 513), then
    G = K' @ W.  The big M-contraction runs as fp8e4 DoubleRow matmuls at 2x
    rate: M is EXACT in fp8 ({0,1,2}) and (a*X2) is fed as an exact hi+lo
    fp8 pair (hi = fp8(ax), lo = fp8(ax - hi), ~7 mantissa bits combined)
    accumulated in one PSUM group -> bf16-grade accuracy, double throughput.
  * Score path computed TRANSPOSED: H^T = W^T X2^T so that the att
    dot-products become tiny PE matvecs (lhsT = att columns) instead of
    128 wide DVE reductions.  lrelu(x) = 0.2x + 0.8relu(x): the relu runs
    on the Activation engine (doubling as the PSUM->SBUF copy); the linear
    0.2 * X2 (W att_j) term is a tiny [N, 2] per-view tensor from the host.
    u,v come back to node-major layout via one transpose-DMA each.
  * Cross-partition max/sum run as GPSIMD partition_all_reduce (no PE
    broadcast matmuls, no round-trip DMAs).
  * z comes from an fp8 a-column contracted d-major (out free = 1, ~free).

Sharding: data-parallel over the 8 (batch b, output view g) pairs, one
NeuronCore each; no cross-core communication.  One SPMD program: all
(b, g)-dependence is baked into per-core input tensors on the host (pure
relayout + sign flips + the tiny l-term).
"""

import sys
from contextlib import ExitStack

for _p in ("/opt/trn_rl_repo/concourse", "/opt/trn_rl_repo"):
    if _p not in sys.path:
        sys.path.insert(0, _p)

import ml_dtypes  # noqa: E402
import numpy as np  # noqa: E402

import concourse.bass as bass  # noqa: E402
import concourse.bacc as bacc  # noqa: E402
import concourse.bass_isa as bass_isa  # noqa: E402
import concourse.mybir as mybir  # noqa: E402
import concourse.tile as tile  # noqa: E402
import concourse.tile_utils as tile_utils  # noqa: E402
import bass_rust  # noqa: E402

# Problem constants (hardcoded per harness contract).
B, V, N, F, O = 2, 4, 2048, 128, 512
NT = N // 128  # node tiles
F2 = 2 * F  # 256 pair features
AXW = 272  # axq row stride (256 feats + a col + pad to %16)
F32, BF16 = mybir.dt.float32, mybir.dt.bfloat16
FP8 = mybir.dt.float8e4
DR = mybir.MatmulPerfMode.DoubleRow
MUL = mybir.AluOpType.mult
ADD = mybir.AluOpType.add
SUB = mybir.AluOpType.subtract
MAX = mybir.AluOpType.max
RELU = mybir.ActivationFunctionType.Relu
EXP = mybir.ActivationFunctionType.Exp

tile_utils.max_sbuf_usage = 207 * 1024

# Skip the lo-residual pass for the first LO_SKIP s-tiles: trades a little
# quantization error (measured 4.3e-3 -> 1.16e-2 at LO_SKIP=2, gate 2e-2)
# for 2/16 of the DoubleRow matmul work.
LO_SKIP = 2


def _chain(f, g):
    if f is None:
        return g
    return lambda: (f(), g())


class _TileContext(tile.TileContext):
    """Splits the exit-drain's sem waits across single-wait carrier nops.

    Walrus caps sync waits at 1/instruction; the stock _drain_and_barrier
    attaches every outstanding DMA/engine sem wait to one Drain and fails
    codegen with "Too many sync wait commands".
    """

    def _drain_and_barrier(self, tick_clock, wait_clock):
        nc = self.nc
        probe = nc.sync.nop(nofuse=True)
        wait_clock.add_sem_waits(
            probe.ins, bass_rust.ScopedClock({None: tick_clock.global_clock})
        )
        si = probe.ins.sync_info
        if si is not None and si.on_wait and len(si.on_wait) > 1:
            waits = list(si.on_wait)
            si.on_wait = [waits[0]]
            for w in waits[1:]:
                carrier = nc.sync.nop(nofuse=True)
                carrier.ins.sync_info = mybir.SyncInfo(on_wait=[w], on_update=[])
        nc.sync.drain()
        nc.all_engine_barrier()
        popped = nc._tile_sem_poison_stack.pop()
        assert popped is self._sem_poison
        nc.clear_and_free_semaphores(list(self.sems.allocated().values()))
        nc.all_engine_barrier()


def _build_program():
    nc = bacc.Bacc("TRN2", target_bir_lowering=False, debug=False)

    sxst_d = nc.dram_tensor("sxst", [V, 128, 2, N], BF16, kind="ExternalInput").ap()
    sxp_d = nc.dram_tensor("sxp", [V, 128, NT * F2], BF16, kind="ExternalInput").ap()
    mt_d = nc.dram_tensor("mt", [4, 128, NT * 512], FP8, kind="ExternalInput").ap()
    wab_d = nc.dram_tensor("wab", [128, 2, O + 8], BF16, kind="ExternalInput").ap()
    lps_d = nc.dram_tensor("lps", [128, V * NT * 2], F32, kind="ExternalInput").ap()
    out_d = nc.dram_tensor("out", [NT, 128, O], F32, kind="ExternalOutput").ap()
    urd_d = nc.dram_tensor("urd", [V, 2, N], F32, kind="Internal").ap()
    import os as _os
    dbg = _os.environ.get("KDBG") == "1"
    if dbg:
        dbg_d = {
            name: nc.dram_tensor(
                name, shape, BF16 if name == "d_kpt" else F32,
                kind="ExternalOutput",
            ).ap()
            for name, shape in {
                "d_urow": [2, N], "d_us": [128, NT], "d_vs": [128, NT],
                "d_a": [128, NT], "d_b": [128, NT], "d_ma": [128, NT],
                "d_kpt": [128, 2 * N], "d_bsc": [128, NT],
            }.items()
        }

    with ExitStack() as ctx:
        tc = ctx.enter_context(_TileContext(nc))
        pool = ctx.enter_context(tc.tile_pool(name="main", bufs=1))
        stp = ctx.enter_context(tc.tile_pool(name="st", bufs=2))
        axp = ctx.enter_context(tc.tile_pool(name="ax", bufs=4))
        ktp = ctx.enter_context(tc.tile_pool(name="kt", bufs=2))
        ltp = ctx.enter_context(tc.tile_pool(name="lt", bufs=8))
        tmp = ctx.enter_context(tc.tile_pool(name="tmq", bufs=2))
        urdp = ctx.enter_context(tc.tile_pool(name="urd", bufs=2, space="DRAM"))
        php = ctx.enter_context(tc.tile_pool(name="ph", bufs=2, space="PSUM"))
        pkp = ctx.enter_context(tc.tile_pool(name="pk", bufs=2, space="PSUM"))
        pgp = ctx.enter_context(tc.tile_pool(name="pg", bufs=3, space="PSUM"))
        pqp = ctx.enter_context(tc.tile_pool(name="pq", bufs=1, space="PSUM"))

        # ---- persistent SBUF ----
        sxst = pool.tile([128, V, 2, N], BF16)  # f-major x^T pairs per view
        sxp = pool.tile([128, V, NT, F2], BF16)  # s-major x pairs per view
        mt = pool.tile([128, NT, N], FP8)  # mt[p, t, d] = M[d, t*128+p]
        wab = pool.tile([128, 2, O + 8], BF16)  # [W half | 0.8*att o-major]
        lps = pool.tile([128, V, NT, 2], F32)  # 0.2 * X2 @ (W att_j)
        out_acc = pool.tile([128, NT, O], F32)

        # Input DMA is STAGED through the pipeline: the cost model (and HW
        # ring occupancy) serializes transfers, so bulk loads are emitted
        # just-in-time to keep the rings free for the small latency-critical
        # stats transpose-DMAs.  Here: only what the prologue needs.
        nc.sync.dma_start(wab[:, 0], wab_d[:, 0])
        nc.sync.dma_start(sxst[:, 0, :, 0:512], sxst_d[0, :, :, 0:512])
        nc.sync.dma_start(wab[:, 1], wab_d[:, 1])
        for c in range(1, 4):  # view-0 f-major pair, chunked so A0 starts early
            cs = slice(c * 512, (c + 1) * 512)
            nc.sync.dma_start(sxst[:, 0, :, cs], sxst_d[0, :, :, cs])
        nc.sync.dma_start(lps[:], lps_d[:])
        nc.sync.dma_start(sxp[:, 0], sxp_d[0])
        nc.sync.dma_start(sxst[:, 1], sxst_d[1])

        def dma_mt(c):
            nc.sync.dma_start(mt[:, :, c * 512 : (c + 1) * 512], mt_d[c])

        def dma_sxst(v):
            nc.sync.dma_start(sxst[:, v], sxst_d[v])

        def dma_sxp(v):
            nc.sync.dma_start(sxp[:, v], sxp_d[v])

        st = {}

        def a_phase(h):
            """H^T = W^T X2^T per s-chunk; relu; u,v via PE matvecs."""
            urow = stp.tile([2, N], F32, tag="urow", name=f"urow{h}")
            rts = {}

            def h_chunk(c):
                for ot in range(4):
                    ph_t = php.tile([128, 512], F32, tag="ph", name=f"ph{h}_{c}_{ot}")
                    for i in range(2):
                        nc.tensor.matmul(
                            ph_t[:], wab[:, i, bass.ts(ot, 128)],
                            sxst[:, h, i, bass.ts(c, 512)],
                            start=(i == 0), stop=(i == 1),
                        )
                    rt = ltp.tile([128, 512], BF16, tag="rt", name=f"rt{h}_{c}_{ot}")
                    if ot % 2 == 1:  # relus split ACT/DVE so PE sets the pace
                        nc.vector.tensor_scalar(rt[:], ph_t[:], 0.0, None, op0=MAX)
                    else:
                        nc.scalar.activation(rt[:], ph_t[:], RELU)
                    rts[(c, ot)] = rt

            def mv_chunk(c):
                pu = pqp.tile([128, 512], F32, tag="pq", name=f"pu{h}_{c}")
                for ot in range(4):
                    nc.tensor.matmul(
                        pu[0:2, :], wab[:, 0:2, O + ot], rts[(c, ot)][:],
                        start=(ot == 0), stop=(ot == 3),
                    )
                nc.scalar.copy(urow[0:2, bass.ts(c, 512)], pu[0:2, :])

            # mv lags two chunks so the relu chain is never on the PE path;
            # the 8-deep rt ring still suffices (rt(c) frees at mv(c)).
            h_chunk(0)
            h_chunk(1)
            h_chunk(2)
            mv_chunk(0)
            mv_chunk(1)
            h_chunk(3)
            mv_chunk(2)
            mv_chunk(3)
            st[h] = [urow]

        def stats_tr(h):
            """u,v back to node-major via transpose-DMA; + linear term;
            global max via partition_all_reduce; a,b = exp(. - max)."""
            urow = st[h][0]
            us = stp.tile([128, NT], F32, tag="us", name=f"us{h}")
            vs = stp.tile([128, NT], F32, tag="vs", name=f"vs{h}")
            # node-major transpose via DRAM bounce (SBUF APs cannot take a
            # partition dim from free strides; DRAM APs are unconstrained).
            # The scratch is a DRAM *pool tile* so the RAW dep is tracked.
            urd_t = urdp.tile([2, N], F32, tag="urd", name=f"urd{h}")
            nc.sync.dma_start(urd_t[:], urow[0:2, :])
            for j, dst in ((0, us), (1, vs)):
                src = urd_t[j : j + 1, :].rearrange(
                    "a (c t p) -> (a p) (c t)", c=4, t=4, p=128
                )
                nc.sync.dma_start(dst[:], src)
                nc.vector.tensor_tensor(dst[:], dst[:], lps[:, h, :, j], op=ADD)
            mstat = stp.tile([128, 2], F32, tag="mst", name=f"mst{h}")
            negm = stp.tile([128, 2], F32, tag="negm", name=f"negm{h}")
            nc.vector.reduce_max(mstat[:, 0:1], us[:], axis=mybir.AxisListType.X)
            nc.vector.reduce_max(mstat[:, 1:2], vs[:], axis=mybir.AxisListType.X)
            nc.gpsimd.partition_all_reduce(
                negm[:], mstat[:], 128, bass_isa.ReduceOp.max
            )
            nc.scalar.mul(negm[:], negm[:], -1.0)
            st[h].extend([us, vs, negm])

        def quant(h, t0, t1):
            """a,b = exp; axq hi/lo fp8 pair of a*X2 for tiles [t0, t1)."""
            if t0 == 0:
                us, vs, negm = st[h][1:4]
                a_st = stp.tile([128, NT], F32, tag="ast", name=f"ast{h}")
                b_st = stp.tile([128, NT], F32, tag="bst", name=f"bst{h}")
                nc.scalar.activation(a_st[:], us[:], EXP, bias=negm[:, 0:1])
                nc.scalar.activation(b_st[:], vs[:], EXP, bias=negm[:, 1:2])
                hi = axp.tile([128, NT, AXW], FP8, tag="hi", name=f"hi{h}")
                lo = axp.tile([128, NT, AXW], FP8, tag="lo", name=f"lo{h}")
                nc.vector.tensor_copy(hi[:, :, F2], a_st[:])  # a col (hi only)
                st[h].extend([a_st, b_st, hi, lo])
            a_st, _, hi, lo = st[h][4:8]
            for t in range(t0, t1):
                if t % 2 == 0:
                    nc.scalar.mul(
                        hi[:, t, 0:F2], sxp[:, h, t, :], a_st[:, t : t + 1]
                    )
                else:
                    nc.vector.tensor_scalar(
                        hi[:, t, 0:F2], sxp[:, h, t, :], a_st[:, t : t + 1],
                        None, op0=MUL,
                    )
            for t in range(max(t0, LO_SKIP), t1):
                nc.vector.scalar_tensor_tensor(
                    lo[:, t, 0:F2], sxp[:, h, t, :], a_st[:, t : t + 1],
                    hi[:, t, 0:F2], op0=MUL, op1=SUB,
                )

        def g_tile(hp, d):
            """One G2 d-tile of view hp: 2 bf16 matmuls + scale-accumulate."""
            kpt, bsc = st[hp][8], st[hp][10]
            pg_t = pgp.tile([128, O], F32, tag="pg", name=f"pg{hp}_{d}")
            nc.tensor.matmul(
                pg_t[:], kpt[:, 0, bass.ts(d, 128)], wab[:, 0, 0:O],
                start=True, stop=False,
            )
            nc.tensor.matmul(
                pg_t[:], kpt[:, 1, bass.ts(d, 128)], wab[:, 1, 0:O],
                start=False, stop=True,
            )
            if hp == 0:
                nc.scalar.mul(out_acc[:, d, :], pg_t[:], bsc[:, d : d + 1])
            elif hp < 3 or d % 2 == 0:
                nc.vector.scalar_tensor_tensor(
                    out_acc[:, d, :], pg_t[:], bsc[:, d : d + 1],
                    out_acc[:, d, :], op0=MUL, op1=ADD,
                )
            else:  # tail: ACT mul + Pool add keeps the epilogue off DVE
                tq = tmp.tile([128, O], BF16, tag="tq", name=f"tq{d}")
                nc.scalar.mul(tq[:], pg_t[:], bsc[:, d : d + 1])
                nc.gpsimd.tensor_tensor(
                    out_acc[:, d, :], tq[:], out_acc[:, d, :], op=ADD
                )
            if hp == V - 1:
                nc.sync.dma_start(out_d[d], out_acc[:, d, :])

        def ma_pass(h, pma, d0, d1):
            """ma columns [d0,d1) as free=1 DoubleRow matvecs (~free on PE).
            One PSUM accumulation group spans all 16 columns of the bank."""
            hi = st[h][6]
            for d in range(d0, d1):
                for t in range(8):
                    nc.tensor.matmul(
                        pma[:, d : d + 1],
                        mt[:, 2 * t : 2 * t + 2, bass.ts(d, 128)],
                        hi[:, 2 * t : 2 * t + 2, F2 : F2 + 1],
                        start=(d == 0 and t == 0), stop=(d == NT - 1 and t == 7),
                        perf_mode=DR, skip_group_check=True,
                    )

        def k_phase(h, hp, hooks=None, ma_first=False, gq2=None):
            """K'^T = M @ (a*X2) via hi+lo fp8 DoubleRow; ma = M @ a8 d-major.
            Interleaves the G2 pass of view hp and per-group hook callbacks."""
            hi, lo = st[h][6], st[h][7]
            kpt = ktp.tile([128, 2, N], BF16, tag="kpt", name=f"kpt{h}")
            ma_sb = stp.tile([128, NT], F32, tag="ma", name=f"ma{h}")
            pma = pqp.tile([128, 512], F32, tag="pq", name=f"pma{h}")
            st[h].append(kpt)
            st[h].append(ma_sb)
            if ma_first:
                ma_pass(h, pma, 0, NT)
                nc.vector.tensor_copy(ma_sb[:], pma[:, 0:NT])
                z_chain(h)
            gq = list(range(NT)) if hp is not None else []
            gi = 0
            for c in range(4):
                for ft in range(2):
                    pk_t = pkp.tile(
                        [128, 512], F32, tag="pk", name=f"pk{h}_{c}_{ft}"
                    )
                    fc = slice(ft * 128, (ft + 1) * 128)
                    dc = slice(c * 512, (c + 1) * 512)
                    for t in range(8):
                        nc.tensor.matmul(
                            pk_t[:], hi[:, 2 * t : 2 * t + 2, fc],
                            mt[:, 2 * t : 2 * t + 2, dc],
                            start=(t == 0), stop=False, perf_mode=DR,
                        )
                    for t in range(LO_SKIP // 2, 8):
                        nc.tensor.matmul(
                            pk_t[:], lo[:, 2 * t : 2 * t + 2, fc],
                            mt[:, 2 * t : 2 * t + 2, dc],
                            start=False, stop=(t == 7), perf_mode=DR,
                        )
                    nc.scalar.copy(kpt[:, ft, dc], pk_t[:])
                    if not ma_first:
                        d = c * 4 + ft * 2
                        ma_pass(h, pma, d, d + 2)
                    if hooks and gi in hooks:
                        hooks[gi]()
                    for _ in range(4 if gq2 is not None else 2):
                        if gq:
                            g_tile(hp, gq.pop(0))
                    # own-view G2 tiles once this chunk's kpt columns landed
                    if gq2 is not None:
                        avail = (c + ft) * 4
                        for _ in range(4):
                            if gq2 and gq2[0] < avail:
                                g_tile(h, gq2.pop(0))
                    gi += 1
            if not ma_first:
                nc.vector.tensor_copy(ma_sb[:], pma[:, 0:NT])

        def z_chain(h):
            """z = sum_d b_d ma_d via partition_all_reduce; bsc = b/(V z)."""
            b_st, ma_sb = st[h][5], st[h][9]
            zscr = stp.tile([128, NT], F32, tag="zscr", name=f"zscr{h}")
            zp = stp.tile([128, 1], F32, tag="zp", name=f"zp{h}")
            zs = stp.tile([128, 1], F32, tag="zs", name=f"zs{h}")
            nc.vector.scalar_tensor_tensor(
                zscr[:], ma_sb[:], 1.0, b_st[:], op0=MUL, op1=MUL,
                accum_out=zp[:],
            )
            nc.gpsimd.partition_all_reduce(
                zs[:], zp[:], 128, bass_isa.ReduceOp.add
            )
            nc.vector.reciprocal(zs[:], zs[:])
            nc.vector.tensor_scalar(zs[:], zs[:], 1.0 / V, None, op0=MUL)
            bsc = stp.tile([128, NT], F32, tag="bsc", name=f"bsc{h}")
            nc.vector.tensor_scalar(bsc[:], b_st[:], zs[:, 0:1], None, op0=MUL)
            st[h].append(bsc)

        def q_hooks(h):
            """Spread view h's exp+quant across the previous k_phase."""
            return {
                0: lambda: quant(h, 0, 6),
                1: lambda: quant(h, 6, 11),
                2: lambda: quant(h, 11, 16),
            }

        # ---- software pipeline over views ----
        # st[h]: 0 urow, 1 us, 2 vs, 3 negm, 4 a, 5 b, 6 hi, 7 lo,
        #        8 kpt, 9 ma, 10 bsc
        a_phase(0)
        stats_tr(0)
        dma_mt(0)
        dma_sxst(2)
        a_phase(1)
        stats_tr(1)
        dma_mt(1)
        dma_sxp(1)
        quant(0, 0, 16)
        k0_hooks = q_hooks(1)
        k0_hooks[1] = _chain(k0_hooks.get(1), lambda: dma_mt(2))
        k0_hooks[3] = _chain(k0_hooks.get(3), lambda: dma_mt(3))
        k0_hooks[5] = _chain(k0_hooks.get(5), lambda: dma_sxp(2))
        k0_hooks[7] = _chain(k0_hooks.get(7), lambda: dma_sxst(3))
        k_phase(0, None, k0_hooks)
        a_phase(2)
        stats_tr(2)
        z_chain(0)
        k1_hooks = q_hooks(2)
        k1_hooks[4] = _chain(k1_hooks.get(4), lambda: dma_sxp(3))
        k_phase(1, 0, k1_hooks)
        a_phase(3)
        stats_tr(3)
        z_chain(1)
        k_phase(2, 1, q_hooks(3))
        z_chain(2)
        gq3 = list(range(NT))
        k_phase(3, 2, ma_first=True, gq2=gq3)
        while gq3:
            g_tile(3, gq3.pop(0))

        if dbg:
            hdbg = 3  # view-3 tiles are still live at program end
            nc.sync.dma_start(dbg_d["d_urow"][:], st[hdbg][0][0:2, :])
            nc.sync.dma_start(dbg_d["d_us"][:], st[hdbg][1][:])
            nc.sync.dma_start(dbg_d["d_vs"][:], st[hdbg][2][:])
            nc.sync.dma_start(dbg_d["d_a"][:], st[hdbg][4][:])
            nc.sync.dma_start(dbg_d["d_b"][:], st[hdbg][5][:])
            nc.sync.dma_start(dbg_d["d_ma"][:], st[hdbg][9][:])
            nc.sync.dma_start(dbg_d["d_bsc"][:], st[hdbg][10][:])
            nc.sync.dma_start(dbg_d["d_kpt"][:], st[hdbg][8][:])

    nc.compile()
    return nc


_SIGNS = None


def _signs():
    global _SIGNS
    if _SIGNS is None:
        s = np.ones((4, F), dtype=np.float32)
        for r in range(4):
            if r & 1:
                s[r, [0, 2]] = -1.0
            if r & 2:
                s[r, [1, 3]] = -1.0
        _SIGNS = s
    return _SIGNS


def _host_prep(x, edge_index, W, att, bias):
    """Per-core relayout + sign flips + the tiny O(N*F) l-term."""
    signs = _signs()
    x = np.ascontiguousarray(x, dtype=np.float32)
    W = np.asarray(W, dtype=np.float32)
    att = np.asarray(att, dtype=np.float32).reshape(2 * O)
    ei = np.asarray(edge_index)

    # mt[p, t, d] = M[d, t*128 + p]; entries {0,1,2} exact in fp8e4
    M = np.zeros((N, N), dtype=np.float32)
    np.add.at(M, (ei[1], ei[0]), 1.0)
    M[np.arange(N), np.arange(N)] += 1.0
    MT = np.ascontiguousarray(M.T).reshape(NT, 128, N).transpose(1, 0, 2)
    mt_t = MT.astype(ml_dtypes.float8_e4m3)  # [128, NT, N]
    mt_chunks = np.ascontiguousarray(
        mt_t.reshape(128, NT, 4, 512).transpose(2, 0, 1, 3).reshape(4, 128, NT * 512)
    )

    # wab[p, j, 0:O] = W[j*128+p, :]; wab[p, j, O+ot] = 0.8*att_j[ot*128+p]
    wab = np.zeros((128, 2, O + 8), dtype=np.float32)
    wab[:, :, :O] = W.reshape(2, 128, O).transpose(1, 0, 2)
    wab[:, 0, O : O + 4] = 0.8 * att[:O].reshape(4, 128).T
    wab[:, 1, O : O + 4] = 0.8 * att[O:].reshape(4, 128).T
    wab = np.ascontiguousarray(wab).astype(ml_dtypes.bfloat16)
    wa = 0.2 * np.stack([W @ att[:O], W @ att[O:]], axis=1)  # [256, 2]

    in_maps = []
    for core in range(8):
        b, g = divmod(core, V)
        # sign-aligned per-view copies: sxs_v = x[b, v] * signs[v ^ g]
        sxs = np.empty((V, N, F), dtype=np.float32)
        for v in range(V):
            sxs[v] = x[b, v] * signs[v ^ g]
        sxst = np.empty((V, 128, 2, N), dtype=np.float32)  # f-major halves
        sxp = np.empty((V, 128, NT, F2), dtype=np.float32)  # s-major pairs
        lps = np.empty((V, 128, NT, 2), dtype=np.float32)  # linear score term
        for h in range(V):
            gh = g ^ h
            sxst[h, :, 0] = sxs[h].T
            sxst[h, :, 1] = sxs[gh].T
            pair = np.concatenate([sxs[h], sxs[gh]], axis=1)  # [N, 256]
            sxp[h] = pair.reshape(NT, 128, F2).transpose(1, 0, 2)
            lps[h] = (pair @ wa).reshape(NT, 128, 2).transpose(1, 0, 2)
        in_maps.append(
            {
                "sxst": sxst.astype(ml_dtypes.bfloat16),
                "sxp": np.ascontiguousarray(
                    sxp.reshape(V, 128, NT * F2)
                ).astype(ml_dtypes.bfloat16),
                "mt": mt_chunks,
                "wab": wab,
                "lps": np.ascontiguousarray(
                    lps.transpose(1, 0, 2, 3).reshape(128, V * NT * 2)
                ),
            }
        )
    return in_maps


_NC = None


def kernel(x, edge_index, W, att, bias):
    global _NC
    if _NC is None:
        _NC = _build_program()
    in_maps = _host_prep(x, edge_index, W, att, bias)

    from concourse.bass_utils import run_bass_kernel_spmd

    res = run_bass_kernel_spmd(_NC, in_maps, list(range(8)))
    out = np.empty((B, V, N, O), dtype=np.float32)
    for core in range(8):
        b, g = divmod(core, V)
        out[b, g] = res.results[core]["out"].reshape(N, O)
    bias = np.asarray(bias, dtype=np.float32)
    if np.any(bias):
        out += bias  # bias is zero for this problem's inputs
    return out


# revision 54
# speedup vs baseline: 1.0380x; 1.0243x over previous
"""Trainium2 Bass kernel for nn_D2GroupConvolutionLayer (D2-equivariant GAT).

Math: for output view g and input view h the layer is a GAT with a GLOBAL
softmax over edges.  Since score(e) = u[src] + v[dst], the pipeline collapses
to dense algebra per (batch, g, h):

    out += diag(b) . M . diag(a) . X2 . W / (V * b^T M a)

with a = exp(u - max u), b = exp(v - max v) per-node scalars, X2 = [x_h, x_gh]
the sign-aligned feature pair [N, 2F], and M[d, s] the {0,1,2} edge
multiplicity matrix (self-loops included).

Key optimizations vs the straightforward M @ (diag(a) X2 W) order:
  * Re-associate: K' = M @ (diag(a) X2) (257-wide rhs instead of 513), then
    G = K' @ W.  The big M-contraction runs as fp8e4 DoubleRow matmuls at 2x
    rate: M is EXACT in fp8 ({0,1,2}) and (a*X2) is fed as an exact hi+lo
    fp8 pair (hi = fp8(ax), lo = fp8(ax - hi), ~7 mantissa bits combined)
    accumulated in one PSUM group -> bf16-grade accuracy, double throughput.
  * Score path computed TRANSPOSED: H^T = W^T X2^T so that the att
    dot-products become tiny PE matvecs (lhsT = att columns) instead of
    128 wide DVE reductions.  lrelu(x) = 0.2x + 0.8relu(x): the relu runs
    on the Activation engine (doubling as the PSUM->SBUF copy); the linear
    0.2 * X2 (W att_j) term is a tiny [N, 2] per-view tensor from the host.
    u,v come back to node-major layout via one transpose-DMA each.
  * Cross-partition max/sum run as GPSIMD partition_all_reduce (no PE
    broadcast matmuls, no round-trip DMAs).
  * z comes from an fp8 a-column contracted d-major (out free = 1, ~free).

Sharding: data-parallel over the 8 (batch b, output view g) pairs, one
NeuronCore each; no cross-core communication.  One SPMD program: all
(b, g)-dependence is baked into per-core input tensors on the host (pure
relayout + sign flips + the tiny l-term).
"""

import sys
from contextlib import ExitStack

for _p in ("/opt/trn_rl_repo/concourse", "/opt/trn_rl_repo"):
    if _p not in sys.path:
        sys.path.insert(0, _p)

import ml_dtypes  # noqa: E402
import numpy as np  # noqa: E402

import concourse.bass as bass  # noqa: E402
import concourse.bacc as bacc  # noqa: E402
import concourse.bass_isa as bass_isa  # noqa: E402
import concourse.mybir as mybir  # noqa: E402
import concourse.tile as tile  # noqa: E402
import concourse.tile_utils as tile_utils  # noqa: E402
import bass_rust  # noqa: E402

# Problem constants (hardcoded per harness contract).
B, V, N, F, O = 2, 4, 2048, 128, 512
NT = N // 128  # node tiles
F2 = 2 * F  # 256 pair features
AXW = 272  # axq row stride (256 feats + a col + pad to %16)
F32, BF16 = mybir.dt.float32, mybir.dt.bfloat16
FP8 = mybir.dt.float8e4
DR = mybir.MatmulPerfMode.DoubleRow
MUL = mybir.AluOpType.mult
ADD = mybir.AluOpType.add
SUB = mybir.AluOpType.subtract
MAX = mybir.AluOpType.max
RELU = mybir.ActivationFunctionType.Relu
EXP = mybir.ActivationFunctionType.Exp

tile_utils.max_sbuf_usage = 207 * 1024

# Skip the lo-residual pass for the first LO_SKIP s-tiles: trades a little
# quantization error (measured 4.3e-3 -> 1.16e-2 at LO_SKIP=2, gate 2e-2)
# for 2/16 of the DoubleRow matmul work.
LO_SKIP = 4


def _chain(f, g):
    if f is None:
        return g
    return lambda: (f(), g())


class _TileContext(tile.TileContext):
    """Splits the exit-drain's sem waits across single-wait carrier nops.

    Walrus caps sync waits at 1/instruction; the stock _drain_and_barrier
    attaches every outstanding DMA/engine sem wait to one Drain and fails
    codegen with "Too many sync wait commands".
    """

    def _drain_and_barrier(self, tick_clock, wait_clock):
        nc = self.nc
        probe = nc.sync.nop(nofuse=True)
        wait_clock.add_sem_waits(
            probe.ins, bass_rust.ScopedClock({None: tick_clock.global_clock})
        )
        si = probe.ins.sync_info
        if si is not None and si.on_wait and len(si.on_wait) > 1:
            waits = list(si.on_wait)
            si.on_wait = [waits[0]]
            for w in waits[1:]:
                carrier = nc.sync.nop(nofuse=True)
                carrier.ins.sync_info = mybir.SyncInfo(on_wait=[w], on_update=[])
        nc.sync.drain()
        nc.all_engine_barrier()
        popped = nc._tile_sem_poison_stack.pop()
        assert popped is self._sem_poison
        nc.clear_and_free_semaphores(list(self.sems.allocated().values()))
        nc.all_engine_barrier()


def _build_program():
    nc = bacc.Bacc("TRN2", target_bir_lowering=False, debug=False)

    sxst_d = nc.dram_tensor("sxst", [V, 128, 2, N], BF16, kind="ExternalInput").ap()
    sxp_d = nc.dram_tensor("sxp", [V, 128, NT * F2], BF16, kind="ExternalInput").ap()
    mt_d = nc.dram_tensor("mt", [4, 128, NT * 512], FP8, kind="ExternalInput").ap()
    wab_d = nc.dram_tensor("wab", [128, 2, O + 8], BF16, kind="ExternalInput").ap()
    lps_d = nc.dram_tensor("lps", [128, V * NT * 2], F32, kind="ExternalInput").ap()
    out_d = nc.dram_tensor("out", [NT, 128, O], F32, kind="ExternalOutput").ap()
    urd_d = nc.dram_tensor("urd", [V, 2, N], F32, kind="Internal").ap()
    import os as _os
    dbg = _os.environ.get("KDBG") == "1"
    if dbg:
        dbg_d = {
            name: nc.dram_tensor(
                name, shape, BF16 if name == "d_kpt" else F32,
                kind="ExternalOutput",
            ).ap()
            for name, shape in {
                "d_urow": [2, N], "d_us": [128, NT], "d_vs": [128, NT],
                "d_a": [128, NT], "d_b": [128, NT], "d_ma": [128, NT],
                "d_kpt": [128, 2 * N], "d_bsc": [128, NT],
            }.items()
        }

    with ExitStack() as ctx:
        tc = ctx.enter_context(_TileContext(nc))
        pool = ctx.enter_context(tc.tile_pool(name="main", bufs=1))
        stp = ctx.enter_context(tc.tile_pool(name="st", bufs=2))
        axp = ctx.enter_context(tc.tile_pool(name="ax", bufs=4))
        ktp = ctx.enter_context(tc.tile_pool(name="kt", bufs=2))
        ltp = ctx.enter_context(tc.tile_pool(name="lt", bufs=8))
        tmp = ctx.enter_context(tc.tile_pool(name="tmq", bufs=2))
        urdp = ctx.enter_context(tc.tile_pool(name="urd", bufs=2, space="DRAM"))
        php = ctx.enter_context(tc.tile_pool(name="ph", bufs=2, space="PSUM"))
        pkp = ctx.enter_context(tc.tile_pool(name="pk", bufs=2, space="PSUM"))
        pgp = ctx.enter_context(tc.tile_pool(name="pg", bufs=3, space="PSUM"))
        pqp = ctx.enter_context(tc.tile_pool(name="pq", bufs=1, space="PSUM"))

        # ---- persistent SBUF ----
        sxst = pool.tile([128, V, 2, N], BF16)  # f-major x^T pairs per view
        sxp = pool.tile([128, V, NT, F2], BF16)  # s-major x pairs per view
        mt = pool.tile([128, NT, N], FP8)  # mt[p, t, d] = M[d, t*128+p]
        wab = pool.tile([128, 2, O + 8], BF16)  # [W half | 0.8*att o-major]
        lps = pool.tile([128, V, NT, 2], F32)  # 0.2 * X2 @ (W att_j)
        out_acc = pool.tile([128, NT, O], F32)

        # Input DMA is STAGED through the pipeline: the cost model (and HW
        # ring occupancy) serializes transfers, so bulk loads are emitted
        # just-in-time to keep the rings free for the small latency-critical
        # stats transpose-DMAs.  Here: only what the prologue needs.
        nc.sync.dma_start(wab[:, 0], wab_d[:, 0])
        nc.sync.dma_start(sxst[:, 0, :, 0:512], sxst_d[0, :, :, 0:512])
        nc.sync.dma_start(wab[:, 1], wab_d[:, 1])
        for c in range(1, 4):  # view-0 f-major pair, chunked so A0 starts early
            cs = slice(c * 512, (c + 1) * 512)
            nc.sync.dma_start(sxst[:, 0, :, cs], sxst_d[0, :, :, cs])
        nc.sync.dma_start(lps[:], lps_d[:])
        nc.sync.dma_start(sxp[:, 0], sxp_d[0])
        nc.sync.dma_start(sxst[:, 1], sxst_d[1])

        def dma_mt(c):
            nc.sync.dma_start(mt[:, :, c * 512 : (c + 1) * 512], mt_d[c])

        def dma_sxst(v):
            nc.sync.dma_start(sxst[:, v], sxst_d[v])

        def dma_sxp(v):
            nc.sync.dma_start(sxp[:, v], sxp_d[v])

        st = {}

        def a_phase(h):
            """H^T = W^T X2^T per s-chunk; relu; u,v via PE matvecs."""
            urow = stp.tile([2, N], F32, tag="urow", name=f"urow{h}")
            rts = {}

            def h_chunk(c):
                for ot in range(4):
                    ph_t = php.tile([128, 512], F32, tag="ph", name=f"ph{h}_{c}_{ot}")
                    for i in range(2):
                        nc.tensor.matmul(
                            ph_t[:], wab[:, i, bass.ts(ot, 128)],
                            sxst[:, h, i, bass.ts(c, 512)],
                            start=(i == 0), stop=(i == 1),
                        )
                    rt = ltp.tile([128, 512], BF16, tag="rt", name=f"rt{h}_{c}_{ot}")
                    if ot % 2 == 1:  # relus split ACT/DVE so PE sets the pace
                        nc.vector.tensor_scalar(rt[:], ph_t[:], 0.0, None, op0=MAX)
                    else:
                        nc.scalar.activation(rt[:], ph_t[:], RELU)
                    rts[(c, ot)] = rt

            def mv_chunk(c):
                pu = pqp.tile([128, 512], F32, tag="pq", name=f"pu{h}_{c}")
                for ot in range(4):
                    nc.tensor.matmul(
                        pu[0:2, :], wab[:, 0:2, O + ot], rts[(c, ot)][:],
                        start=(ot == 0), stop=(ot == 3),
                    )
                nc.scalar.copy(urow[0:2, bass.ts(c, 512)], pu[0:2, :])

            # mv lags two chunks so the relu chain is never on the PE path;
            # the 8-deep rt ring still suffices (rt(c) frees at mv(c)).
            h_chunk(0)
            h_chunk(1)
            h_chunk(2)
            mv_chunk(0)
            mv_chunk(1)
            h_chunk(3)
            mv_chunk(2)
            mv_chunk(3)
            st[h] = [urow]

        def stats_tr(h):
            """u,v back to node-major via transpose-DMA; + linear term;
            global max via partition_all_reduce; a,b = exp(. - max)."""
            urow = st[h][0]
            us = stp.tile([128, NT], F32, tag="us", name=f"us{h}")
            vs = stp.tile([128, NT], F32, tag="vs", name=f"vs{h}")
            # node-major transpose via DRAM bounce (SBUF APs cannot take a
            # partition dim from free strides; DRAM APs are unconstrained).
            # The scratch is a DRAM *pool tile* so the RAW dep is tracked.
            urd_t = urdp.tile([2, N], F32, tag="urd", name=f"urd{h}")
            nc.sync.dma_start(urd_t[:], urow[0:2, :])
            for j, dst in ((0, us), (1, vs)):
                src = urd_t[j : j + 1, :].rearrange(
                    "a (c t p) -> (a p) (c t)", c=4, t=4, p=128
                )
                nc.sync.dma_start(dst[:], src)
                nc.vector.tensor_tensor(dst[:], dst[:], lps[:, h, :, j], op=ADD)
            mstat = stp.tile([128, 2], F32, tag="mst", name=f"mst{h}")
            negm = stp.tile([128, 2], F32, tag="negm", name=f"negm{h}")
            nc.vector.reduce_max(mstat[:, 0:1], us[:], axis=mybir.AxisListType.X)
            nc.vector.reduce_max(mstat[:, 1:2], vs[:], axis=mybir.AxisListType.X)
            nc.gpsimd.partition_all_reduce(
                negm[:], mstat[:], 128, bass_isa.ReduceOp.max
            )
            nc.scalar.mul(negm[:], negm[:], -1.0)
            st[h].extend([us, vs, negm])

        def quant(h, t0, t1):
            """a,b = exp; axq hi/lo fp8 pair of a*X2 for tiles [t0, t1)."""
            if t0 == 0:
                us, vs, negm = st[h][1:4]
                a_st = stp.tile([128, NT], F32, tag="ast", name=f"ast{h}")
                b_st = stp.tile([128, NT], F32, tag="bst", name=f"bst{h}")
                nc.scalar.activation(a_st[:], us[:], EXP, bias=negm[:, 0:1])
                nc.scalar.activation(b_st[:], vs[:], EXP, bias=negm[:, 1:2])
                hi = axp.tile([128, NT, AXW], FP8, tag="hi", name=f"hi{h}")
                lo = axp.tile([128, NT, AXW], FP8, tag="lo", name=f"lo{h}")
                nc.vector.tensor_copy(hi[:, :, F2], a_st[:])  # a col (hi only)
                st[h].extend([a_st, b_st, hi, lo])
            a_st, _, hi, lo = st[h][4:8]
            for t in range(t0, t1):
                if t % 2 == 0:
                    nc.scalar.mul(
                        hi[:, t, 0:F2], sxp[:, h, t, :], a_st[:, t : t + 1]
                    )
                else:
                    nc.vector.tensor_scalar(
                        hi[:, t, 0:F2], sxp[:, h, t, :], a_st[:, t : t + 1],
                        None, op0=MUL,
                    )
            for t in range(max(t0, LO_SKIP), t1):
                nc.vector.scalar_tensor_tensor(
                    lo[:, t, 0:F2], sxp[:, h, t, :], a_st[:, t : t + 1],
                    hi[:, t, 0:F2], op0=MUL, op1=SUB,
                )

        def g_tile(hp, d):
            """One G2 d-tile of view hp: 2 bf16 matmuls + scale-accumulate."""
            kpt, bsc = st[hp][8], st[hp][10]
            pg_t = pgp.tile([128, O], F32, tag="pg", name=f"pg{hp}_{d}")
            nc.tensor.matmul(
                pg_t[:], kpt[:, 0, bass.ts(d, 128)], wab[:, 0, 0:O],
                start=True, stop=False,
            )
            nc.tensor.matmul(
                pg_t[:], kpt[:, 1, bass.ts(d, 128)], wab[:, 1, 0:O],
                start=False, stop=True,
            )
            if hp == 0:
                nc.scalar.mul(out_acc[:, d, :], pg_t[:], bsc[:, d : d + 1])
            elif hp < 3 or d % 2 == 0:
                nc.vector.scalar_tensor_tensor(
                    out_acc[:, d, :], pg_t[:], bsc[:, d : d + 1],
                    out_acc[:, d, :], op0=MUL, op1=ADD,
                )
            else:  # tail: ACT mul + Pool add keeps the epilogue off DVE
                tq = tmp.tile([128, O], BF16, tag="tq", name=f"tq{d}")
                nc.scalar.mul(tq[:], pg_t[:], bsc[:, d : d + 1])
                nc.gpsimd.tensor_tensor(
                    out_acc[:, d, :], tq[:], out_acc[:, d, :], op=ADD
                )
            if hp == V - 1:
                nc.sync.dma_start(out_d[d], out_acc[:, d, :])

        def ma_pass(h, pma, d0, d1):
            """ma columns [d0,d1) as free=1 DoubleRow matvecs (~free on PE).
            One PSUM accumulation group spans all 16 columns of the bank."""
            hi = st[h][6]
            for d in range(d0, d1):
                for t in range(8):
                    nc.tensor.matmul(
                        pma[:, d : d + 1],
                        mt[:, 2 * t : 2 * t + 2, bass.ts(d, 128)],
                        hi[:, 2 * t : 2 * t + 2, F2 : F2 + 1],
                        start=(d == 0 and t == 0), stop=(d == NT - 1 and t == 7),
                        perf_mode=DR, skip_group_check=True,
                    )

        def k_phase(h, hp, hooks=None, ma_first=False, gq2=None):
            """K'^T = M @ (a*X2) via hi+lo fp8 DoubleRow; ma = M @ a8 d-major.
            Interleaves the G2 pass of view hp and per-group hook callbacks."""
            hi, lo = st[h][6], st[h][7]
            kpt = ktp.tile([128, 2, N], BF16, tag="kpt", name=f"kpt{h}")
            ma_sb = stp.tile([128, NT], F32, tag="ma", name=f"ma{h}")
            pma = pqp.tile([128, 512], F32, tag="pq", name=f"pma{h}")
            st[h].append(kpt)
            st[h].append(ma_sb)
            if ma_first:
                ma_pass(h, pma, 0, NT)
                nc.vector.tensor_copy(ma_sb[:], pma[:, 0:NT])
                z_chain(h)
            gq = list(range(NT)) if hp is not None else []
            gi = 0
            for c in range(4):
                for ft in range(2):
                    pk_t = pkp.tile(
                        [128, 512], F32, tag="pk", name=f"pk{h}_{c}_{ft}"
                    )
                    fc = slice(ft * 128, (ft + 1) * 128)
                    dc = slice(c * 512, (c + 1) * 512)
                    for t in range(8):
                        nc.tensor.matmul(
                            pk_t[:], hi[:, 2 * t : 2 * t + 2, fc],
                            mt[:, 2 * t : 2 * t + 2, dc],
                            start=(t == 0), stop=False, perf_mode=DR,
                        )
                    for t in range(LO_SKIP // 2, 8):
                        nc.tensor.matmul(
                            pk_t[:], lo[:, 2 * t : 2 * t + 2, fc],
                            mt[:, 2 * t : 2 * t + 2, dc],
                            start=False, stop=(t == 7), perf_mode=DR,
                        )
                    nc.scalar.copy(kpt[:, ft, dc], pk_t[:])
                    if not ma_first:
                        d = c * 4 + ft * 2
                        ma_pass(h, pma, d, d + 2)
                    if hooks and gi in hooks:
                        hooks[gi]()
                    for _ in range(4 if gq2 is not None else 2):
                        if gq:
                            g_tile(hp, gq.pop(0))
                    # own-view G2 tiles once this chunk's kpt columns landed
                    if gq2 is not None:
                        avail = (c + ft) * 4
                        for _ in range(4):
                            if gq2 and gq2[0] < avail:
                                g_tile(h, gq2.pop(0))
                    gi += 1
            if not ma_first:
                nc.vector.tensor_copy(ma_sb[:], pma[:, 0:NT])

        def z_chain(h):
            """z = sum_d b_d ma_d via partition_all_reduce; bsc = b/(V z)."""
            b_st, ma_sb = st[h][5], st[h][9]
            zscr = stp.tile([128, NT], F32, tag="zscr", name=f"zscr{h}")
            zp = stp.tile([128, 1], F32, tag="zp", name=f"zp{h}")
            zs = stp.tile([128, 1], F32, tag="zs", name=f"zs{h}")
            nc.vector.scalar_tensor_tensor(
                zscr[:], ma_sb[:], 1.0, b_st[:], op0=MUL, op1=MUL,
                accum_out=zp[:],
            )
            nc.gpsimd.partition_all_reduce(
                zs[:], zp[:], 128, bass_isa.ReduceOp.add
            )
            nc.vector.reciprocal(zs[:], zs[:])
            nc.vector.tensor_scalar(zs[:], zs[:], 1.0 / V, None, op0=MUL)
            bsc = stp.tile([128, NT], F32, tag="bsc", name=f"bsc{h}")
            nc.vector.tensor_scalar(bsc[:], b_st[:], zs[:, 0:1], None, op0=MUL)
            st[h].append(bsc)

        def q_hooks(h):
            """Spread view h's exp+quant across the previous k_phase."""
            return {
                0: lambda: quant(h, 0, 6),
                1: lambda: quant(h, 6, 11),
                2: lambda: quant(h, 11, 16),
            }

        # ---- software pipeline over views ----
        # st[h]: 0 urow, 1 us, 2 vs, 3 negm, 4 a, 5 b, 6 hi, 7 lo,
        #        8 kpt, 9 ma, 10 bsc
        a_phase(0)
        stats_tr(0)
        dma_mt(0)
        dma_sxst(2)
        a_phase(1)
        stats_tr(1)
        dma_mt(1)
        dma_sxp(1)
        quant(0, 0, 16)
        k0_hooks = q_hooks(1)
        k0_hooks[1] = _chain(k0_hooks.get(1), lambda: dma_mt(2))
        k0_hooks[3] = _chain(k0_hooks.get(3), lambda: dma_mt(3))
        k0_hooks[5] = _chain(k0_hooks.get(5), lambda: dma_sxp(2))
        k0_hooks[7] = _chain(k0_hooks.get(7), lambda: dma_sxst(3))
        k_phase(0, None, k0_hooks)
        a_phase(2)
        stats_tr(2)
        z_chain(0)
        k1_hooks = q_hooks(2)
        k1_hooks[4] = _chain(k1_hooks.get(4), lambda: dma_sxp(3))
        k_phase(1, 0, k1_hooks)
        a_phase(3)
        stats_tr(3)
        z_chain(1)
        k_phase(2, 1, q_hooks(3))
        z_chain(2)
        gq3 = list(range(NT))
        k_phase(3, 2, ma_first=True, gq2=gq3)
        while gq3:
            g_tile(3, gq3.pop(0))

        if dbg:
            hdbg = 3  # view-3 tiles are still live at program end
            nc.sync.dma_start(dbg_d["d_urow"][:], st[hdbg][0][0:2, :])
            nc.sync.dma_start(dbg_d["d_us"][:], st[hdbg][1][:])
            nc.sync.dma_start(dbg_d["d_vs"][:], st[hdbg][2][:])
            nc.sync.dma_start(dbg_d["d_a"][:], st[hdbg][4][:])
            nc.sync.dma_start(dbg_d["d_b"][:], st[hdbg][5][:])
            nc.sync.dma_start(dbg_d["d_ma"][:], st[hdbg][9][:])
            nc.sync.dma_start(dbg_d["d_bsc"][:], st[hdbg][10][:])
            nc.sync.dma_start(dbg_d["d_kpt"][:], st[hdbg][8][:])

    nc.compile()
    return nc


_SIGNS = None


def _signs():
    global _SIGNS
    if _SIGNS is None:
        s = np.ones((4, F), dtype=np.float32)
        for r in range(4):
            if r & 1:
                s[r, [0, 2]] = -1.0
            if r & 2:
                s[r, [1, 3]] = -1.0
        _SIGNS = s
    return _SIGNS


def _host_prep(x, edge_index, W, att, bias):
    """Per-core relayout + sign flips + the tiny O(N*F) l-term."""
    signs = _signs()
    x = np.ascontiguousarray(x, dtype=np.float32)
    W = np.asarray(W, dtype=np.float32)
    att = np.asarray(att, dtype=np.float32).reshape(2 * O)
    ei = np.asarray(edge_index)

    # mt[p, t, d] = M[d, t*128 + p]; entries {0,1,2} exact in fp8e4
    M = np.zeros((N, N), dtype=np.float32)
    np.add.at(M, (ei[1], ei[0]), 1.0)
    M[np.arange(N), np.arange(N)] += 1.0
    MT = np.ascontiguousarray(M.T).reshape(NT, 128, N).transpose(1, 0, 2)
    mt_t = MT.astype(ml_dtypes.float8_e4m3)  # [128, NT, N]
    mt_chunks = np.ascontiguousarray(
        mt_t.reshape(128, NT, 4, 512).transpose(2, 0, 1, 3).reshape(4, 128, NT * 512)
    )

    # wab[p, j, 0:O] = W[j*128+p, :]; wab[p, j, O+ot] = 0.8*att_j[ot*128+p]
    wab = np.zeros((128, 2, O + 8), dtype=np.float32)
    wab[:, :, :O] = W.reshape(2, 128, O).transpose(1, 0, 2)
    wab[:, 0, O : O + 4] = 0.8 * att[:O].reshape(4, 128).T
    wab[:, 1, O : O + 4] = 0.8 * att[O:].reshape(4, 128).T
    wab = np.ascontiguousarray(wab).astype(ml_dtypes.bfloat16)
    wa = 0.2 * np.stack([W @ att[:O], W @ att[O:]], axis=1)  # [256, 2]

    in_maps = []
    for core in range(8):
        b, g = divmod(core, V)
        # sign-aligned per-view copies: sxs_v = x[b, v] * signs[v ^ g]
        sxs = np.empty((V, N, F), dtype=np.float32)
        for v in range(V):
            sxs[v] = x[b, v] * signs[v ^ g]
        sxst = np.empty((V, 128, 2, N), dtype=np.float32)  # f-major halves
        sxp = np.empty((V, 128, NT, F2), dtype=np.float32)  # s-major pairs
        lps = np.empty((V, 128, NT, 2), dtype=np.float32)  # linear score term
        for h in range(V):
            gh = g ^ h
            sxst[h, :, 0] = sxs[h].T
            sxst[h, :, 1] = sxs[gh].T
            pair = np.concatenate([sxs[h], sxs[gh]], axis=1)  # [N, 256]
            sxp[h] = pair.reshape(NT, 128, F2).transpose(1, 0, 2)
            lps[h] = (pair @ wa).reshape(NT, 128, 2).transpose(1, 0, 2)
        in_maps.append(
            {
                "sxst": sxst.astype(ml_dtypes.bfloat16),
                "sxp": np.ascontiguousarray(
                    sxp.reshape(V, 128, NT * F2)
                ).astype(ml_dtypes.bfloat16),
                "mt": mt_chunks,
                "wab": wab,
                "lps": np.ascontiguousarray(
                    lps.transpose(1, 0, 2, 3).reshape(128, V * NT * 2)
                ),
            }
        )
    return in_maps


_NC = None


def kernel(x, edge_index, W, att, bias):
    global _NC
    if _NC is None:
        _NC = _build_program()
    in_maps = _host_prep(x, edge_index, W, att, bias)

    from concourse.bass_utils import run_bass_kernel_spmd

    res = run_bass_kernel_spmd(_NC, in_maps, list(range(8)))
    out = np.empty((B, V, N, O), dtype=np.float32)
    for core in range(8):
        b, g = divmod(core, V)
        out[b, g] = res.results[core]["out"].reshape(N, O)
    bias = np.asarray(bias, dtype=np.float32)
    if np.any(bias):
        out += bias  # bias is zero for this problem's inputs
    return out


# revision 55
# speedup vs baseline: 1.0518x; 1.0133x over previous
"""Trainium2 Bass kernel for nn_D2GroupConvolutionLayer (D2-equivariant GAT).

Math: for output view g and input view h the layer is a GAT with a GLOBAL
softmax over edges.  Since score(e) = u[src] + v[dst], the pipeline collapses
to dense algebra per (batch, g, h):

    out += diag(b) . M . diag(a) . X2 . W / (V * b^T M a)

with a = exp(u - max u), b = exp(v - max v) per-node scalars, X2 = [x_h, x_gh]
the sign-aligned feature pair [N, 2F], and M[d, s] the {0,1,2} edge
multiplicity matrix (self-loops included).

Key optimizations vs the straightforward M @ (diag(a) X2 W) order:
  * Re-associate: K' = M @ (diag(a) X2) (257-wide rhs instead of 513), then
    G = K' @ W.  The big M-contraction runs as fp8e4 DoubleRow matmuls at 2x
    rate: M is EXACT in fp8 ({0,1,2}) and (a*X2) is fed as an exact hi+lo
    fp8 pair (hi = fp8(ax), lo = fp8(ax - hi), ~7 mantissa bits combined)
    accumulated in one PSUM group -> bf16-grade accuracy, double throughput.
  * Score path computed TRANSPOSED: H^T = W^T X2^T so that the att
    dot-products become tiny PE matvecs (lhsT = att columns) instead of
    128 wide DVE reductions.  lrelu(x) = 0.2x + 0.8relu(x): the relu runs
    on the Activation engine (doubling as the PSUM->SBUF copy); the linear
    0.2 * X2 (W att_j) term is a tiny [N, 2] per-view tensor from the host.
    u,v come back to node-major layout via one transpose-DMA each.
  * Cross-partition max/sum run as GPSIMD partition_all_reduce (no PE
    broadcast matmuls, no round-trip DMAs).
  * z comes from an fp8 a-column contracted d-major (out free = 1, ~free).

Sharding: data-parallel over the 8 (batch b, output view g) pairs, one
NeuronCore each; no cross-core communication.  One SPMD program: all
(b, g)-dependence is baked into per-core input tensors on the host (pure
relayout + sign flips + the tiny l-term).
"""

import sys
from contextlib import ExitStack

for _p in ("/opt/trn_rl_repo/concourse", "/opt/trn_rl_repo"):
    if _p not in sys.path:
        sys.path.insert(0, _p)

import ml_dtypes  # noqa: E402
import numpy as np  # noqa: E402

import concourse.bass as bass  # noqa: E402
import concourse.bacc as bacc  # noqa: E402
import concourse.bass_isa as bass_isa  # noqa: E402
import concourse.mybir as mybir  # noqa: E402
import concourse.tile as tile  # noqa: E402
import concourse.tile_utils as tile_utils  # noqa: E402
import bass_rust  # noqa: E402

# Problem constants (hardcoded per harness contract).
B, V, N, F, O = 2, 4, 2048, 128, 512
NT = N // 128  # node tiles
F2 = 2 * F  # 256 pair features
AXW = 272  # axq row stride (256 feats + a col + pad to %16)
F32, BF16 = mybir.dt.float32, mybir.dt.bfloat16
FP8 = mybir.dt.float8e4
DR = mybir.MatmulPerfMode.DoubleRow
MUL = mybir.AluOpType.mult
ADD = mybir.AluOpType.add
SUB = mybir.AluOpType.subtract
MAX = mybir.AluOpType.max
RELU = mybir.ActivationFunctionType.Relu
EXP = mybir.ActivationFunctionType.Exp

tile_utils.max_sbuf_usage = 207 * 1024

# Skip the lo-residual pass for the first LO_SKIP s-tiles: trades a little
# quantization error (measured 4.3e-3 -> 1.16e-2 at LO_SKIP=2, gate 2e-2)
# for 2/16 of the DoubleRow matmul work.
LO_SKIP = 4


def _chain(f, g):
    if f is None:
        return g
    return lambda: (f(), g())


class _TileContext(tile.TileContext):
    """Splits the exit-drain's sem waits across single-wait carrier nops.

    Walrus caps sync waits at 1/instruction; the stock _drain_and_barrier
    attaches every outstanding DMA/engine sem wait to one Drain and fails
    codegen with "Too many sync wait commands".
    """

    def _drain_and_barrier(self, tick_clock, wait_clock):
        nc = self.nc
        probe = nc.sync.nop(nofuse=True)
        wait_clock.add_sem_waits(
            probe.ins, bass_rust.ScopedClock({None: tick_clock.global_clock})
        )
        si = probe.ins.sync_info
        if si is not None and si.on_wait and len(si.on_wait) > 1:
            waits = list(si.on_wait)
            si.on_wait = [waits[0]]
            for w in waits[1:]:
                carrier = nc.sync.nop(nofuse=True)
                carrier.ins.sync_info = mybir.SyncInfo(on_wait=[w], on_update=[])
        nc.sync.drain()
        nc.all_engine_barrier()
        popped = nc._tile_sem_poison_stack.pop()
        assert popped is self._sem_poison
        nc.clear_and_free_semaphores(list(self.sems.allocated().values()))
        nc.all_engine_barrier()


def _build_program():
    nc = bacc.Bacc("TRN2", target_bir_lowering=False, debug=False)

    sxst_d = nc.dram_tensor("sxst", [V, 128, 2, N], BF16, kind="ExternalInput").ap()
    sxp_d = nc.dram_tensor("sxp", [V, 128, NT * F2], BF16, kind="ExternalInput").ap()
    mt_d = nc.dram_tensor("mt", [4, 128, NT * 512], FP8, kind="ExternalInput").ap()
    wab_d = nc.dram_tensor("wab", [128, 2, O + 8], BF16, kind="ExternalInput").ap()
    lps_d = nc.dram_tensor("lps", [128, V * NT * 2], F32, kind="ExternalInput").ap()
    out_d = nc.dram_tensor("out", [NT, 128, O], F32, kind="ExternalOutput").ap()
    urd_d = nc.dram_tensor("urd", [V, 2, N], F32, kind="Internal").ap()
    import os as _os
    dbg = _os.environ.get("KDBG") == "1"
    if dbg:
        dbg_d = {
            name: nc.dram_tensor(
                name, shape, BF16 if name == "d_kpt" else F32,
                kind="ExternalOutput",
            ).ap()
            for name, shape in {
                "d_urow": [2, N], "d_us": [128, NT], "d_vs": [128, NT],
                "d_a": [128, NT], "d_b": [128, NT], "d_ma": [128, NT],
                "d_kpt": [128, 2 * N], "d_bsc": [128, NT],
            }.items()
        }

    with ExitStack() as ctx:
        tc = ctx.enter_context(_TileContext(nc))
        pool = ctx.enter_context(tc.tile_pool(name="main", bufs=1))
        stp = ctx.enter_context(tc.tile_pool(name="st", bufs=2))
        axp = ctx.enter_context(tc.tile_pool(name="ax", bufs=4))
        ktp = ctx.enter_context(tc.tile_pool(name="kt", bufs=2))
        ltp = ctx.enter_context(tc.tile_pool(name="lt", bufs=8))
        tmp = ctx.enter_context(tc.tile_pool(name="tmq", bufs=2))
        urdp = ctx.enter_context(tc.tile_pool(name="urd", bufs=2, space="DRAM"))
        php = ctx.enter_context(tc.tile_pool(name="ph", bufs=2, space="PSUM"))
        pkp = ctx.enter_context(tc.tile_pool(name="pk", bufs=2, space="PSUM"))
        pgp = ctx.enter_context(tc.tile_pool(name="pg", bufs=2, space="PSUM"))
        pqp = ctx.enter_context(tc.tile_pool(name="pq", bufs=2, space="PSUM"))

        # ---- persistent SBUF ----
        sxst = pool.tile([128, V, 2, N], BF16)  # f-major x^T pairs per view
        sxp = pool.tile([128, V, NT, F2], BF16)  # s-major x pairs per view
        mt = pool.tile([128, NT, N], FP8)  # mt[p, t, d] = M[d, t*128+p]
        wab = pool.tile([128, 2, O + 8], BF16)  # [W half | 0.8*att o-major]
        lps = pool.tile([128, V, NT, 2], F32)  # 0.2 * X2 @ (W att_j)
        out_acc = pool.tile([128, NT, O], F32)

        # Input DMA is STAGED through the pipeline: the cost model (and HW
        # ring occupancy) serializes transfers, so bulk loads are emitted
        # just-in-time to keep the rings free for the small latency-critical
        # stats transpose-DMAs.  Here: only what the prologue needs.
        nc.sync.dma_start(wab[:, 0], wab_d[:, 0])
        nc.sync.dma_start(sxst[:, 0, :, 0:512], sxst_d[0, :, :, 0:512])
        nc.sync.dma_start(wab[:, 1], wab_d[:, 1])
        for c in range(1, 4):  # view-0 f-major pair, chunked so A0 starts early
            cs = slice(c * 512, (c + 1) * 512)
            nc.sync.dma_start(sxst[:, 0, :, cs], sxst_d[0, :, :, cs])
        nc.sync.dma_start(lps[:], lps_d[:])
        nc.sync.dma_start(sxp[:, 0], sxp_d[0])
        nc.sync.dma_start(sxst[:, 1], sxst_d[1])

        def dma_mt(c):
            nc.sync.dma_start(mt[:, :, c * 512 : (c + 1) * 512], mt_d[c])

        def dma_sxst(v):
            nc.sync.dma_start(sxst[:, v], sxst_d[v])

        def dma_sxp(v):
            nc.sync.dma_start(sxp[:, v], sxp_d[v])

        st = {}

        def a_phase(h):
            """H^T = W^T X2^T per s-chunk; relu; u,v via PE matvecs."""
            urow = stp.tile([2, N], F32, tag="urow", name=f"urow{h}")
            rts = {}

            def h_chunk(c):
                for ot in range(4):
                    ph_t = php.tile([128, 512], F32, tag="ph", name=f"ph{h}_{c}_{ot}")
                    for i in range(2):
                        nc.tensor.matmul(
                            ph_t[:], wab[:, i, bass.ts(ot, 128)],
                            sxst[:, h, i, bass.ts(c, 512)],
                            start=(i == 0), stop=(i == 1),
                        )
                    rt = ltp.tile([128, 512], BF16, tag="rt", name=f"rt{h}_{c}_{ot}")
                    if ot % 2 == 1:  # relus split ACT/DVE so PE sets the pace
                        nc.vector.tensor_scalar(rt[:], ph_t[:], 0.0, None, op0=MAX)
                    else:
                        nc.scalar.activation(rt[:], ph_t[:], RELU)
                    rts[(c, ot)] = rt

            def mv_chunk(c):
                pu = pqp.tile([128, 512], F32, tag="pq", name=f"pu{h}_{c}")
                for ot in range(4):
                    nc.tensor.matmul(
                        pu[0:2, :], wab[:, 0:2, O + ot], rts[(c, ot)][:],
                        start=(ot == 0), stop=(ot == 3),
                    )
                nc.scalar.copy(urow[0:2, bass.ts(c, 512)], pu[0:2, :])

            # mv lags two chunks so the relu chain is never on the PE path;
            # the 8-deep rt ring still suffices (rt(c) frees at mv(c)).
            h_chunk(0)
            h_chunk(1)
            h_chunk(2)
            mv_chunk(0)
            mv_chunk(1)
            h_chunk(3)
            mv_chunk(2)
            mv_chunk(3)
            st[h] = [urow]

        def stats_tr(h):
            """u,v back to node-major via transpose-DMA; + linear term;
            global max via partition_all_reduce; a,b = exp(. - max)."""
            urow = st[h][0]
            us = stp.tile([128, NT], F32, tag="us", name=f"us{h}")
            vs = stp.tile([128, NT], F32, tag="vs", name=f"vs{h}")
            # node-major transpose via DRAM bounce (SBUF APs cannot take a
            # partition dim from free strides; DRAM APs are unconstrained).
            # The scratch is a DRAM *pool tile* so the RAW dep is tracked.
            urd_t = urdp.tile([2, N], F32, tag="urd", name=f"urd{h}")
            nc.sync.dma_start(urd_t[:], urow[0:2, :])
            for j, dst in ((0, us), (1, vs)):
                src = urd_t[j : j + 1, :].rearrange(
                    "a (c t p) -> (a p) (c t)", c=4, t=4, p=128
                )
                nc.sync.dma_start(dst[:], src)
                nc.vector.tensor_tensor(dst[:], dst[:], lps[:, h, :, j], op=ADD)
            mstat = stp.tile([128, 2], F32, tag="mst", name=f"mst{h}")
            negm = stp.tile([128, 2], F32, tag="negm", name=f"negm{h}")
            nc.vector.reduce_max(mstat[:, 0:1], us[:], axis=mybir.AxisListType.X)
            nc.vector.reduce_max(mstat[:, 1:2], vs[:], axis=mybir.AxisListType.X)
            nc.gpsimd.partition_all_reduce(
                negm[:], mstat[:], 128, bass_isa.ReduceOp.max
            )
            nc.scalar.mul(negm[:], negm[:], -1.0)
            st[h].extend([us, vs, negm])

        def quant(h, t0, t1):
            """a,b = exp; axq hi/lo fp8 pair of a*X2 for tiles [t0, t1)."""
            if t0 == 0:
                us, vs, negm = st[h][1:4]
                a_st = stp.tile([128, NT], F32, tag="ast", name=f"ast{h}")
                b_st = stp.tile([128, NT], F32, tag="bst", name=f"bst{h}")
                nc.scalar.activation(a_st[:], us[:], EXP, bias=negm[:, 0:1])
                nc.scalar.activation(b_st[:], vs[:], EXP, bias=negm[:, 1:2])
                hi = axp.tile([128, NT, AXW], FP8, tag="hi", name=f"hi{h}")
                lo = axp.tile([128, NT, AXW], FP8, tag="lo", name=f"lo{h}")
                nc.vector.tensor_copy(hi[:, :, F2], a_st[:])  # a col (hi only)
                st[h].extend([a_st, b_st, hi, lo])
            a_st, _, hi, lo = st[h][4:8]
            for t in range(t0, t1):
                if t % 2 == 0:
                    nc.scalar.mul(
                        hi[:, t, 0:F2], sxp[:, h, t, :], a_st[:, t : t + 1]
                    )
                else:
                    nc.vector.tensor_scalar(
                        hi[:, t, 0:F2], sxp[:, h, t, :], a_st[:, t : t + 1],
                        None, op0=MUL,
                    )
            for t in range(max(t0, LO_SKIP), t1):
                nc.vector.scalar_tensor_tensor(
                    lo[:, t, 0:F2], sxp[:, h, t, :], a_st[:, t : t + 1],
                    hi[:, t, 0:F2], op0=MUL, op1=SUB,
                )

        def g_tile(hp, d):
            """One G2 d-tile of view hp: 2 bf16 matmuls + scale-accumulate."""
            kpt, bsc = st[hp][8], st[hp][10]
            pg_t = pgp.tile([128, O], F32, tag="pg", name=f"pg{hp}_{d}")
            nc.tensor.matmul(
                pg_t[:], kpt[:, 0, bass.ts(d, 128)], wab[:, 0, 0:O],
                start=True, stop=False,
            )
            nc.tensor.matmul(
                pg_t[:], kpt[:, 1, bass.ts(d, 128)], wab[:, 1, 0:O],
                start=False, stop=True,
            )
            if hp == 0:
                nc.scalar.mul(out_acc[:, d, :], pg_t[:], bsc[:, d : d + 1])
            elif hp < 3 or d % 2 == 0:
                nc.vector.scalar_tensor_tensor(
                    out_acc[:, d, :], pg_t[:], bsc[:, d : d + 1],
                    out_acc[:, d, :], op0=MUL, op1=ADD,
                )
            else:  # tail: ACT mul + Pool add keeps the epilogue off DVE
                tq = tmp.tile([128, O], BF16, tag="tq", name=f"tq{d}")
                nc.scalar.mul(tq[:], pg_t[:], bsc[:, d : d + 1])
                nc.gpsimd.tensor_tensor(
                    out_acc[:, d, :], tq[:], out_acc[:, d, :], op=ADD
                )
            if hp == V - 1:
                nc.sync.dma_start(out_d[d], out_acc[:, d, :])

        def ma_pass(h, pma, d0, d1):
            """ma columns [d0,d1) as free=1 DoubleRow matvecs (~free on PE).
            One PSUM accumulation group spans all 16 columns of the bank."""
            hi = st[h][6]
            for d in range(d0, d1):
                for t in range(8):
                    nc.tensor.matmul(
                        pma[:, d : d + 1],
                        mt[:, 2 * t : 2 * t + 2, bass.ts(d, 128)],
                        hi[:, 2 * t : 2 * t + 2, F2 : F2 + 1],
                        start=(d == 0 and t == 0), stop=(d == NT - 1 and t == 7),
                        perf_mode=DR, skip_group_check=True,
                    )

        def k_phase(h, hp, hooks=None, ma_first=False, gq2=None):
            """K'^T = M @ (a*X2) via hi+lo fp8 DoubleRow; ma = M @ a8 d-major.
            Interleaves the G2 pass of view hp and per-group hook callbacks."""
            hi, lo = st[h][6], st[h][7]
            kpt = ktp.tile([128, 2, N], BF16, tag="kpt", name=f"kpt{h}")
            ma_sb = stp.tile([128, NT], F32, tag="ma", name=f"ma{h}")
            pma = pqp.tile([128, 512], F32, tag="pq", name=f"pma{h}")
            st[h].append(kpt)
            st[h].append(ma_sb)
            if ma_first:
                ma_pass(h, pma, 0, NT)
                nc.vector.tensor_copy(ma_sb[:], pma[:, 0:NT])
                z_chain(h)
            gq = list(range(NT)) if hp is not None else []
            gi = 0
            for c in range(4):
                for ft in range(2):
                    pk_t = pkp.tile(
                        [128, 512], F32, tag="pk", name=f"pk{h}_{c}_{ft}"
                    )
                    fc = slice(ft * 128, (ft + 1) * 128)
                    dc = slice(c * 512, (c + 1) * 512)
                    for t in range(8):
                        nc.tensor.matmul(
                            pk_t[:], hi[:, 2 * t : 2 * t + 2, fc],
                            mt[:, 2 * t : 2 * t + 2, dc],
                            start=(t == 0), stop=False, perf_mode=DR,
                        )
                    for t in range(LO_SKIP // 2, 8):
                        nc.tensor.matmul(
                            pk_t[:], lo[:, 2 * t : 2 * t + 2, fc],
                            mt[:, 2 * t : 2 * t + 2, dc],
                            start=False, stop=(t == 7), perf_mode=DR,
                        )
                    nc.scalar.copy(kpt[:, ft, dc], pk_t[:])
                    if not ma_first:
                        d = c * 4 + ft * 2
                        ma_pass(h, pma, d, d + 2)
                    if hooks and gi in hooks:
                        hooks[gi]()
                    for _ in range(4 if gq2 is not None else 2):
                        if gq:
                            g_tile(hp, gq.pop(0))
                    # own-view G2 tiles once this chunk's kpt columns landed
                    if gq2 is not None:
                        avail = (c + ft) * 4
                        for _ in range(4):
                            if gq2 and gq2[0] < avail:
                                g_tile(h, gq2.pop(0))
                    gi += 1
            if not ma_first:
                nc.vector.tensor_copy(ma_sb[:], pma[:, 0:NT])

        def z_chain(h):
            """z = sum_d b_d ma_d via partition_all_reduce; bsc = b/(V z)."""
            b_st, ma_sb = st[h][5], st[h][9]
            zscr = stp.tile([128, NT], F32, tag="zscr", name=f"zscr{h}")
            zp = stp.tile([128, 1], F32, tag="zp", name=f"zp{h}")
            zs = stp.tile([128, 1], F32, tag="zs", name=f"zs{h}")
            nc.vector.scalar_tensor_tensor(
                zscr[:], ma_sb[:], 1.0, b_st[:], op0=MUL, op1=MUL,
                accum_out=zp[:],
            )
            nc.gpsimd.partition_all_reduce(
                zs[:], zp[:], 128, bass_isa.ReduceOp.add
            )
            nc.vector.reciprocal(zs[:], zs[:])
            nc.vector.tensor_scalar(zs[:], zs[:], 1.0 / V, None, op0=MUL)
            bsc = stp.tile([128, NT], F32, tag="bsc", name=f"bsc{h}")
            nc.vector.tensor_scalar(bsc[:], b_st[:], zs[:, 0:1], None, op0=MUL)
            st[h].append(bsc)

        def q_hooks(h):
            """Spread view h's exp+quant across the previous k_phase."""
            return {
                0: lambda: quant(h, 0, 6),
                1: lambda: quant(h, 6, 11),
                2: lambda: quant(h, 11, 16),
            }

        # ---- software pipeline over views ----
        # st[h]: 0 urow, 1 us, 2 vs, 3 negm, 4 a, 5 b, 6 hi, 7 lo,
        #        8 kpt, 9 ma, 10 bsc
        a_phase(0)
        stats_tr(0)
        dma_mt(0)
        dma_sxst(2)
        a_phase(1)
        stats_tr(1)
        dma_mt(1)
        dma_sxp(1)
        quant(0, 0, 16)
        k0_hooks = q_hooks(1)
        k0_hooks[1] = _chain(k0_hooks.get(1), lambda: dma_mt(2))
        k0_hooks[3] = _chain(k0_hooks.get(3), lambda: dma_mt(3))
        k0_hooks[5] = _chain(k0_hooks.get(5), lambda: dma_sxp(2))
        k0_hooks[7] = _chain(k0_hooks.get(7), lambda: dma_sxst(3))
        k_phase(0, None, k0_hooks)
        a_phase(2)
        stats_tr(2)
        z_chain(0)
        k1_hooks = q_hooks(2)
        k1_hooks[4] = _chain(k1_hooks.get(4), lambda: dma_sxp(3))
        k_phase(1, 0, k1_hooks)
        a_phase(3)
        stats_tr(3)
        z_chain(1)
        k_phase(2, 1, q_hooks(3))
        z_chain(2)
        gq3 = list(range(NT))
        k_phase(3, 2, ma_first=True, gq2=gq3)
        while gq3:
            g_tile(3, gq3.pop(0))

        if dbg:
            hdbg = 3  # view-3 tiles are still live at program end
            nc.sync.dma_start(dbg_d["d_urow"][:], st[hdbg][0][0:2, :])
            nc.sync.dma_start(dbg_d["d_us"][:], st[hdbg][1][:])
            nc.sync.dma_start(dbg_d["d_vs"][:], st[hdbg][2][:])
            nc.sync.dma_start(dbg_d["d_a"][:], st[hdbg][4][:])
            nc.sync.dma_start(dbg_d["d_b"][:], st[hdbg][5][:])
            nc.sync.dma_start(dbg_d["d_ma"][:], st[hdbg][9][:])
            nc.sync.dma_start(dbg_d["d_bsc"][:], st[hdbg][10][:])
            nc.sync.dma_start(dbg_d["d_kpt"][:], st[hdbg][8][:])

    nc.compile()
    return nc


_SIGNS = None


def _signs():
    global _SIGNS
    if _SIGNS is None:
        s = np.ones((4, F), dtype=np.float32)
        for r in range(4):
            if r & 1:
                s[r, [0, 2]] = -1.0
            if r & 2:
                s[r, [1, 3]] = -1.0
        _SIGNS = s
    return _SIGNS


def _host_prep(x, edge_index, W, att, bias):
    """Per-core relayout + sign flips + the tiny O(N*F) l-term."""
    signs = _signs()
    x = np.ascontiguousarray(x, dtype=np.float32)
    W = np.asarray(W, dtype=np.float32)
    att = np.asarray(att, dtype=np.float32).reshape(2 * O)
    ei = np.asarray(edge_index)

    # mt[p, t, d] = M[d, t*128 + p]; entries {0,1,2} exact in fp8e4
    M = np.zeros((N, N), dtype=np.float32)
    np.add.at(M, (ei[1], ei[0]), 1.0)
    M[np.arange(N), np.arange(N)] += 1.0
    MT = np.ascontiguousarray(M.T).reshape(NT, 128, N).transpose(1, 0, 2)
    mt_t = MT.astype(ml_dtypes.float8_e4m3)  # [128, NT, N]
    mt_chunks = np.ascontiguousarray(
        mt_t.reshape(128, NT, 4, 512).transpose(2, 0, 1, 3).reshape(4, 128, NT * 512)
    )

    # wab[p, j, 0:O] = W[j*128+p, :]; wab[p, j, O+ot] = 0.8*att_j[ot*128+p]
    wab = np.zeros((128, 2, O + 8), dtype=np.float32)
    wab[:, :, :O] = W.reshape(2, 128, O).transpose(1, 0, 2)
    wab[:, 0, O : O + 4] = 0.8 * att[:O].reshape(4, 128).T
    wab[:, 1, O : O + 4] = 0.8 * att[O:].reshape(4, 128).T
    wab = np.ascontiguousarray(wab).astype(ml_dtypes.bfloat16)
    wa = 0.2 * np.stack([W @ att[:O], W @ att[O:]], axis=1)  # [256, 2]

    in_maps = []
    for core in range(8):
        b, g = divmod(core, V)
        # sign-aligned per-view copies: sxs_v = x[b, v] * signs[v ^ g]
        sxs = np.empty((V, N, F), dtype=np.float32)
        for v in range(V):
            sxs[v] = x[b, v] * signs[v ^ g]
        sxst = np.empty((V, 128, 2, N), dtype=np.float32)  # f-major halves
        sxp = np.empty((V, 128, NT, F2), dtype=np.float32)  # s-major pairs
        lps = np.empty((V, 128, NT, 2), dtype=np.float32)  # linear score term
        for h in range(V):
            gh = g ^ h
            sxst[h, :, 0] = sxs[h].T
            sxst[h, :, 1] = sxs[gh].T
            pair = np.concatenate([sxs[h], sxs[gh]], axis=1)  # [N, 256]
            sxp[h] = pair.reshape(NT, 128, F2).transpose(1, 0, 2)
            lps[h] = (pair @ wa).reshape(NT, 128, 2).transpose(1, 0, 2)
        in_maps.append(
            {
                "sxst": sxst.astype(ml_dtypes.bfloat16),
                "sxp": np.ascontiguousarray(
                    sxp.reshape(V, 128, NT * F2)
                ).astype(ml_dtypes.bfloat16),
                "mt": mt_chunks,
                "wab": wab,
                "lps": np.ascontiguousarray(
                    lps.transpose(1, 0, 2, 3).reshape(128, V * NT * 2)
                ),
            }
        )
    return in_maps


_NC = None


def kernel(x, edge_index, W, att, bias):
    global _NC
    if _NC is None:
        _NC = _build_program()
    in_maps = _host_prep(x, edge_index, W, att, bias)

    from concourse.bass_utils import run_bass_kernel_spmd

    res = run_bass_kernel_spmd(_NC, in_maps, list(range(8)))
    out = np.empty((B, V, N, O), dtype=np.float32)
    for core in range(8):
        b, g = divmod(core, V)
        out[b, g] = res.results[core]["out"].reshape(N, O)
    bias = np.asarray(bias, dtype=np.float32)
    if np.any(bias):
        out += bias  # bias is zero for this problem's inputs
    return out
